# revision 1
# baseline (speedup 1.0000x reference)
"""Trainium2 Bass kernel for nn_GAT_78546361909763.

Computes, per sample b (B=16, N=2048, D=256):
    s_j = x @ w[:D];  s_i = x @ w[D:]
    att[i,j] = s_i[i] + s_j[j]
    att = LayerNorm_{(N,N)}(att) * gamma + beta    (gamma==1, beta==0 fast path)
    att = LeakyReLU_{0.2}(att)
    att = softmax(att, axis=-2)                     (normalize each column j over i)
    out = sigmoid(att @ x)

Key algebraic facts exploited on-device:
  * LayerNorm stats over the (N,N) matrix decompose: mean = mean(s_i)+mean(s_j),
    var = var(s_i)+var(s_j), so stats come from the two (N,) vectors.
  * exp(leaky(z)) with z = r*(s+c) equals exp(r*(max(s, 0.2*s - 0.8*c) + c)),
    i.e. one DVE scalar_tensor_tensor + one ACT Exp (with per-partition
    bias r*c and scale r) per tile.
  * The softmax denominator depends only on the contraction index j, so it
    folds into x:  out[i,d] = sum_j expT[j,i] * (x[j,d] / den[j]).
  * max-subtraction in softmax is skipped: |z| <= ~6 so exp never overflows,
    and softmax is shift-invariant.
  * sigmoid(y) = 0.5 + 0.5*tanh(y/2); Tanh and Exp share one ACT table set.

Layout: att is built transposed (j on partitions, i on the free axis) so the
softmax reduction is a free-axis accumulation (free via ACT accum_out) and the
final matmul out_T[d,i] = sum_j xt[j,d] * expT[j,i] contracts j on partitions.
The kernel emits out_T (B, D, N); the host transposes back.

Sharding: data-parallel over B across 8 cores (2 samples per core).
"""

import sys

sys.path.insert(0, "/opt/trn_rl_repo")

import numpy as np

import concourse.bass as bass
import concourse.tile as tile
from concourse import bacc, bass_isa, mybir
from concourse.bass_utils import run_bass_kernel_spmd

B, N, D = 16, 2048, 256
NCORES = 8
BL = B // NCORES            # samples per core
NCH = N // 128              # 16 row chunks of 128
NEG = 0.2                   # leaky relu slope
EPS = 1e-14
FP = mybir.dt.float32
BF = mybir.dt.bfloat16
AF = mybir.ActivationFunctionType
ALU = mybir.AluOpType


def _emit_rsqrt(nc, pool, v_ap):
    """r = 1/sqrt(v + EPS) on DVE only (avoids ACT table switches).

    Fast inverse sqrt seed + 3 Newton iterations on a [128,1] f32 tile.
    """
    vv = pool.tile([128, 1], FP, tag="nwt_vv")
    nc.vector.tensor_scalar(vv[:, :], v_ap, float(EPS), None, ALU.add)
    # seed: y0 = bitcast(0x5f3759df - (bitcast(vv) >> 1))
    yi = pool.tile([128, 1], mybir.dt.int32, tag="nwt_yi")
    nc.vector.tensor_scalar(yi[:, :], vv[:, :].bitcast(mybir.dt.int32), 1, None,
                            ALU.arith_shift_right)
    # y0i = MAGIC - (vi >> 1), as (-1)*(vi>>1) + MAGIC (arith-only ops)
    nc.vector.tensor_scalar(yi[:, :], yi[:, :], -1, 0x5F3759DF,
                            ALU.mult, ALU.add)
    y = pool.tile([128, 1], FP, tag="nwt_y")
    nc.vector.tensor_copy(y[:, :], yi[:, :].bitcast(FP))
    t = pool.tile([128, 1], FP, tag="nwt_t")
    for _ in range(3):
        nc.vector.tensor_tensor(t[:, :], y[:, :], y[:, :], ALU.mult)
        nc.vector.tensor_tensor(t[:, :], t[:, :], vv[:, :], ALU.mult)
        nc.vector.tensor_scalar(t[:, :], t[:, :], -0.5, 1.5, ALU.mult, ALU.add)
        nc.vector.tensor_tensor(y[:, :], y[:, :], t[:, :], ALU.mult)
    return y


def _emit_kernel(tc, out_d, x_d, w_d, reps=1):
    # python-unrolled reps (used only for timing amplification; a For_i
    # device loop wedges the exec unit on this runtime)
    for _ in range(reps):
        _emit_body(tc, out_d, x_d, w_d)


def _emit_body(tc, out_d, x_d, w_d):
    nc = tc.nc
    ctxs = []

    def mkpool(name, bufs, **kw):
        p = tc.alloc_tile_pool(name=name, bufs=bufs, **kw)
        ctxs.append(p)
        return p

    consts = mkpool("consts", 1)
    px = mkpool("px", 24)            # x chunks, f32 [128,256]
    pscr = mkpool("pscr", 2)         # matvec product scratch
    psmall = mkpool("psmall", 2)     # per-sample small tiles
    pnwt = mkpool("pnwt", 2)         # newton temps
    prepl = mkpool("prepl", 2)       # s_repl / s02_repl
    prow = mkpool("prow", 2)         # [1, N] gather row
    pv0 = mkpool("pv0", 5)           # build tiles bf16 [128, N]
    pexp = mkpool("pexp", 7)         # exp tiles bf16 [128, N]
    pxt = mkpool("pxt", 6)           # x~ chunks bf16 [128, D]
    pstg = mkpool("pstg", 3)         # output staging f32 [128, N]
    ppsum = mkpool("ppsum", 2, space="PSUM")
    pdram = mkpool("pdram", 2, space="DRAM")

    w_sb = consts.tile([128, 2 * D], FP)
    nc.sync.dma_start(w_sb[:, :], w_d[:, :])
    zero = consts.tile([128, 1], FP)
    nc.vector.memset(zero[:, :], 0.0)

    xch = {}          # (s, c) -> x chunk tile
    MATVEC_GPS = False

    def emit_xload(s, c):
        xt_ = px.tile([128, D], FP, tag="xchunk", name=f"x_{s}_{c}")
        nc.sync.dma_start(xt_[:, :], x_d[s, c * 128:(c + 1) * 128, :])
        xch[(s, c)] = xt_

    def emit_matvec(s, c, stats_in):
        for h in range(2):
            scr = pscr.tile([128, D], FP, tag="scr", name=f"scr_{s}_{c}_{h}")
            eng = nc.gpsimd if MATVEC_GPS else nc.vector
            eng.scalar_tensor_tensor(
                scr[:, :], xch[(s, c)][:, :], 0.0,
                w_sb[:, h * D:(h + 1) * D],
                ALU.bypass, ALU.mult,
                accum_out=stats_in[:, h * NCH + c:h * NCH + c + 1],
            )

    def emit_stats_repl(s, stats_in):
        nc.vector.tensor_tensor(stats_in[:, 2 * NCH:], stats_in[:, :2 * NCH],
                                stats_in[:, :2 * NCH], ALU.mult)
        sums4 = psmall.tile([128, 4], FP, tag="sums4", name=f"sums4_{s}")
        nc.vector.tensor_reduce(
            sums4[:, :],
            stats_in[:, :].rearrange("p (g c) -> p g c", g=4),
            mybir.AxisListType.X, ALU.add)
        tot4 = psmall.tile([128, 4], FP, tag="tot4", name=f"tot4_{s}")
        nc.gpsimd.partition_all_reduce(tot4[:, :], sums4[:, :], 128,
                                       bass_isa.ReduceOp.add)
        mean4 = psmall.tile([128, 4], FP, tag="mean4", name=f"mean4_{s}")
        nc.vector.tensor_scalar(mean4[:, :], tot4[:, :], 1.0 / N, None, ALU.mult)
        m = psmall.tile([128, 1], FP, tag="m", name=f"m_{s}")
        nc.vector.tensor_tensor(m[:, :], mean4[:, 0:1], mean4[:, 1:2], ALU.add)
        msq = psmall.tile([128, 2], FP, tag="msq", name=f"msq_{s}")
        nc.vector.tensor_tensor(msq[:, :], mean4[:, 0:2], mean4[:, 0:2], ALU.mult)
        q = psmall.tile([128, 1], FP, tag="q", name=f"q_{s}")
        nc.vector.tensor_tensor(q[:, :], mean4[:, 2:3], mean4[:, 3:4], ALU.add)
        m2 = psmall.tile([128, 1], FP, tag="m2", name=f"m2_{s}")
        nc.vector.tensor_tensor(m2[:, :], msq[:, 0:1], msq[:, 1:2], ALU.add)
        v = psmall.tile([128, 1], FP, tag="v", name=f"v_{s}")
        nc.vector.tensor_tensor(v[:, :], q[:, :], m2[:, :], ALU.subtract)
        r = _emit_rsqrt(nc, pnwt, v[:, :])
        cc = psmall.tile([128, NCH], FP, tag="cc", name=f"cc_{s}")
        nc.vector.tensor_scalar(cc[:, :], stats_in[:, 0:NCH], m[:, 0:1], None,
                                ALU.subtract)
        nb08 = psmall.tile([128, NCH], FP, tag="nb08", name=f"nb08_{s}")
        nc.vector.tensor_scalar(nb08[:, :], cc[:, :], -(1.0 - NEG), None, ALU.mult)
        rc = psmall.tile([128, NCH], FP, tag="rc", name=f"rc_{s}")
        nc.vector.tensor_scalar(rc[:, :], cc[:, :], r[:, 0:1], None, ALU.mult)
        # s_i columns -> row via 4 DVE 32x32 block transposes
        si_bf = psmall.tile([128, 32], BF, tag="si_bf", name=f"si_bf_{s}")
        nc.vector.memset(si_bf[:, NCH:], 0.0)
        nc.vector.tensor_copy(si_bf[:, 0:NCH], stats_in[:, NCH:2 * NCH])
        rowt = psmall.tile([32, 128], BF, tag="rowt", name=f"rowt_{s}")
        for b in range(4):
            nc.vector.transpose(rowt[0:32, b * 32:(b + 1) * 32],
                                si_bf[b * 32:(b + 1) * 32, :])
        dlin = pdram.tile([NCH, 128], BF, tag="dlin", name=f"dlin_{s}")
        nc.sync.dma_start(dlin[:, :], rowt[0:NCH, :])
        row = prow.tile([1, N], BF, tag="row", name=f"row_{s}")
        for a in range(NCH):
            nc.sync.dma_start(row[0:1, a * 128:(a + 1) * 128], dlin[a:a + 1, :])
        s_repl = prepl.tile([128, N], BF, tag="s_repl", name=f"s_repl_{s}")
        nc.gpsimd.partition_broadcast(s_repl[:, :], row[:, :])
        s02 = prepl.tile([128, N], BF, tag="s02", name=f"s02_{s}")
        nc.vector.tensor_scalar(s02[:, :], s_repl[:, :], NEG, None, ALU.mult)
        return dict(r=r, rc=rc, nb08=nb08, s_repl=s_repl, s02=s02)

    # ---- startup: sample 0 prologue ----
    st_in = {0: psmall.tile([128, 4 * NCH], FP, tag="stats_in", name="si0")}
    for c in range(NCH):
        emit_xload(0, c)
    for c in range(NCH):
        emit_matvec(0, c, st_in[0])
    state = {0: emit_stats_repl(0, st_in[0])}

    for s in range(BL):
        po = [ppsum.tile([128, N], FP, tag="po", name=f"po_{s}_{d}")
              for d in range(2)]
        den = psmall.tile([128, NCH], FP, tag="den", name=f"den_{s}")
        dinv = psmall.tile([128, NCH], FP, tag="dinv", name=f"dinv_{s}")
        stv = state[s]
        if s + 1 < BL:
            st_in[s + 1] = psmall.tile([128, 4 * NCH], FP, tag="stats_in",
                                       name=f"si{s + 1}")
        for c in range(NCH):
            # ---- interleaved next-sample prologue ----
            if s + 1 < BL:
                if c < 8:
                    emit_xload(s + 1, 2 * c)
                    emit_xload(s + 1, 2 * c + 1)
                    emit_matvec(s + 1, 2 * c, st_in[s + 1])
                    emit_matvec(s + 1, 2 * c + 1, st_in[s + 1])
                elif c == 8:
                    state[s + 1] = emit_stats_repl(s + 1, st_in[s + 1])
            # ---- chunk build ----
            v0a = pv0.tile([128, N], BF, tag="v0a", name=f"v0a_{s}_{c}")
            nc.vector.tensor_scalar(v0a[:, :], stv["s02"][:, :],
                                    stv["nb08"][:, c:c + 1], None, ALU.add)
            v0 = pv0.tile([128, N], BF, tag="v0", name=f"v0_{s}_{c}")
            nc.vector.tensor_tensor(v0[:, :], v0a[:, :], stv["s_repl"][:, :],
                                    ALU.max)
            et = pexp.tile([128, N], BF, tag="exp", name=f"et_{s}_{c}")
            nc.scalar.activation(
                et[:, :], v0[:, :], AF.Exp,
                bias=stv["rc"][:, c:c + 1], scale=stv["r"][:, 0:1],
                accum_out=den[:, c:c + 1])
            if c % 2 == 0:
                et_prev = et
            else:
                nc.vector.reciprocal(dinv[:, c - 1:c + 1], den[:, c - 1:c + 1])
                for cc_, et_use in ((c - 1, et_prev), (c, et)):
                    xt = pxt.tile([128, D], BF, tag="xt", name=f"xt_{s}_{cc_}")
                    nc.gpsimd.tensor_scalar(xt[:, :], xch.pop((s, cc_))[:, :],
                                            dinv[:, cc_:cc_ + 1], None, ALU.mult)
                    for d in range(2):
                        for nn in range(4):
                            nc.tensor.matmul(
                                po[d][:, nn * 512:(nn + 1) * 512],
                                xt[:, d * 128:(d + 1) * 128],
                                et_use[:, nn * 512:(nn + 1) * 512],
                                start=(cc_ == 0), stop=(cc_ == NCH - 1))

        # ---- drain: sigmoid(y) = 0.5 + 0.5*tanh(y/2) ----
        for d in range(2):
            stg = pstg.tile([128, N], FP, tag="stg", name=f"stg_{s}_{d}")
            nc.scalar.activation(stg[:, :], po[d][:, :], AF.Tanh,
                                 bias=zero[:, 0:1], scale=0.5)
            nc.gpsimd.tensor_scalar(stg[:, :], stg[:, :], 0.5, 0.5,
                                    ALU.mult, ALU.add)
            nc.sync.dma_start(out_d[s, d * 128:(d + 1) * 128, :], stg[:, :])

    for p in reversed(ctxs):
        p.release()


_NC = {}


def _get_nc(reps=1):
    if reps not in _NC:
        nc = bacc.Bacc("TRN2", target_bir_lowering=False, debug=False,
                       enable_asserts=False, num_devices=NCORES)
        x_d = nc.dram_tensor("x", [BL, N, D], FP, kind="ExternalInput").ap()
        w_d = nc.dram_tensor("w", [128, 2 * D], FP, kind="ExternalInput").ap()
        out_d = nc.dram_tensor("out_t", [BL, D, N], FP, kind="ExternalOutput").ap()
        with tile.TileContext(nc) as tc:
            _emit_kernel(tc, out_d, x_d, w_d, reps=reps)
        nc.compile()
        _NC[reps] = nc
    return _NC[reps]


def _numpy_fallback(x, weight, gamma, beta):
    out = np.empty((x.shape[0], x.shape[1], x.shape[2]), np.float32)
    d = x.shape[-1]
    for b in range(x.shape[0]):
        xb = x[b].astype(np.float64)
        s_j = xb @ weight[:d].astype(np.float64)
        s_i = xb @ weight[d:].astype(np.float64)
        att = s_i[:, None] + s_j[None, :]
        mean = att.mean()
        var = ((att - mean) ** 2).mean()
        att = (att - mean) / np.sqrt(var + EPS) * gamma + beta
        att = np.where(att >= 0, att, NEG * att)
        att = att - att.max(axis=0, keepdims=True)
        e = np.exp(att)
        att = e / e.sum(axis=0, keepdims=True)
        out[b] = 1.0 / (1.0 + np.exp(-(att @ xb)))
    return out


def run(inputs, trace=False):
    """Run the device kernel. Returns (output, exec_time_ns or None)."""
    x = np.ascontiguousarray(np.asarray(inputs["x"], dtype=np.float32))
    w = np.asarray(inputs["weight"], dtype=np.float32)
    w_repl = np.ascontiguousarray(np.broadcast_to(w, (128, 2 * D)))
    nc = _get_nc()
    in_maps = [
        {"x": np.ascontiguousarray(x[i * BL:(i + 1) * BL]), "w": w_repl}
        for i in range(NCORES)
    ]
    try:
        res = run_bass_kernel_spmd(nc, in_maps, core_ids=list(range(NCORES)),
                                   trace=trace)
    except ModuleNotFoundError:
        res = run_bass_kernel_spmd(nc, in_maps, core_ids=list(range(NCORES)),
                                   trace=False)
    parts = [np.transpose(res.results[i]["out_t"], (0, 2, 1))
             for i in range(NCORES)]
    out = np.concatenate(parts, axis=0)
    return out, res.exec_time_ns


def kernel(**inputs):
    gamma = np.asarray(inputs["gamma"])
    beta = np.asarray(inputs["beta"])
    if not (np.all(gamma == 1.0) and np.all(beta == 0.0)):
        return _numpy_fallback(
            np.asarray(inputs["x"], np.float32),
            np.asarray(inputs["weight"], np.float32),
            gamma.astype(np.float32), beta.astype(np.float32))
    out, _ = run(inputs)
    return out



# revision 21
# speedup vs baseline: 1.1449x; 1.1449x over previous
"""Trainium2 Bass kernel for nn_GAT_78546361909763.

Computes, per sample b (B=16, N=2048, D=256):
    s_j = x @ w[:D];  s_i = x @ w[D:]
    att[i,j] = s_i[i] + s_j[j]
    att = LayerNorm_{(N,N)}(att) * gamma + beta    (gamma==1, beta==0 fast path)
    att = LeakyReLU_{0.2}(att)
    att = softmax(att, axis=-2)                     (normalize each column j over i)
    out = sigmoid(att @ x)

Algebraic structure exploited on-device:
  * LayerNorm stats over the (N,N) matrix decompose: mean = mean(s_i)+mean(s_j),
    var = var(s_i)+var(s_j), so stats come from the two (N,) vectors.
  * exp(leaky(z)) with z = r*(s+c) equals exp(r*(max(s, 0.2*s - 0.8*c) + c)):
    one DVE tensor_scalar + one tensor_tensor max + one ACT Exp per tile.
  * Factorized alternative (no ACT): exp(r*max(a,b)) = max(exp(ra), exp(rb)),
    so et = max(B_j*A_i, Bt_j*At_i) with A = exp(r*s_i) etc. — two DVE
    tensor_scalar multiplies + one tensor_tensor_reduce max (accumulating den).
  * The softmax denominator depends only on the contraction index j, so it
    folds into x:  out[i,d] = sum_j expT[j,i] * (x[j,d] / den[j]).
  * Per-j scale factors cancel between et and den, so each chunk may carry an
    arbitrary exponent shift: a constant -SHIFT keeps exp() inside fp8e4m3
    range, enabling fp8 DoubleRow matmuls (2 j-chunks contracted per pass).
  * fp8 stores truncate; pre-scaling by F8CORR=1+2^-5 centers the error.

Layout: att is built transposed (j on partitions, i on the free axis): the
softmax reduction is a free-axis accumulation (ACT accum_out / TTR accum) and
out_T[d,i] = sum_j xt[j,d] * expT[j,i] contracts j on partitions. The kernel
emits out_T (BL, D, N); the host transposes back.

Sharding: data-parallel over B across 8 cores (2 samples per core).
"""

import sys

sys.path.insert(0, "/opt/trn_rl_repo")

import math

import numpy as np

import concourse.bass as bass
import concourse.tile as tile
from concourse import bacc, bass_isa, mybir
from concourse.bass_utils import run_bass_kernel_spmd

B, N, D = 16, 2048, 256
NCORES = 8
BL = B // NCORES            # samples per core
NCH = N // 128              # 16 row chunks of 128
NEG = 0.2                   # leaky relu slope
EPS = 1e-14
FP = mybir.dt.float32
BF = mybir.dt.bfloat16
F8 = mybir.dt.float8e4
AF = mybir.ActivationFunctionType
ALU = mybir.AluOpType
PM = mybir.MatmulPerfMode

BIAS_CONST = 0.0            # bf16 et needs no range shift
K_B = 0                     # factorized (no-ACT) chunks per sample (at the end)
NA = NCH - K_B
POOL_V0A = frozenset((7, 9, 11, 13))   # chunks whose v0a build runs on Pool


def _emit_rsqrt(nc, pool, v_ap):
    """r = 1/sqrt(v + EPS) on DVE only: fast-inverse-sqrt seed + 3 Newton."""
    vv = pool.tile([128, 1], FP, tag="nwt_vv")
    nc.vector.tensor_scalar(vv[:, :], v_ap, float(EPS), None, ALU.add)
    yi = pool.tile([128, 1], mybir.dt.int32, tag="nwt_yi")
    nc.vector.tensor_scalar(yi[:, :], vv[:, :].bitcast(mybir.dt.int32), 1, None,
                            ALU.arith_shift_right)
    nc.vector.tensor_scalar(yi[:, :], yi[:, :], -1, 0x5F3759DF,
                            ALU.mult, ALU.add)
    y = pool.tile([128, 1], FP, tag="nwt_y")
    nc.vector.tensor_copy(y[:, :], yi[:, :].bitcast(FP))
    t = pool.tile([128, 1], FP, tag="nwt_t")
    for _ in range(3):
        nc.vector.tensor_tensor(t[:, :], y[:, :], y[:, :], ALU.mult)
        nc.vector.tensor_tensor(t[:, :], t[:, :], vv[:, :], ALU.mult)
        nc.vector.tensor_scalar(t[:, :], t[:, :], -0.5, 1.5, ALU.mult, ALU.add)
        nc.vector.tensor_tensor(y[:, :], y[:, :], t[:, :], ALU.mult)
    return y


def _emit_body(tc, out_d, x_d, w_d):
    nc = tc.nc
    ctxs = []

    def mkpool(name, bufs, **kw):
        p = tc.alloc_tile_pool(name=name, bufs=bufs, **kw)
        ctxs.append(p)
        return p

    consts = mkpool("consts", 1)
    px = mkpool("px", 4)             # x group tiles f32 [128, 4*D]
    pscr = mkpool("pscr", 8)         # matvec product scratch
    psmall = mkpool("psmall", 2)     # per-sample small tiles
    pnwt = mkpool("pnwt", 2)         # newton temps
    prepl = mkpool("prepl", 2)       # s_repl / A_repl / At_repl
    prow = mkpool("prow", 2)         # transpose staging
    pv0 = mkpool("pv0", 7)           # build tiles bf16 [128, N]
    pv0p = mkpool("pv0p", 3)         # Pool-built v0a lookahead tiles
    pet = mkpool("pet", 8)           # exp tiles bf16 [128, N]
    pxt = mkpool("pxt", 4)           # x~ tiles bf16 [128, D]
    pstg = mkpool("pstg", 2)         # output staging f32 [128, N]
    ppsum = mkpool("ppsum", 2, space="PSUM")
    pdram = mkpool("pdram", 2, space="DRAM")

    w_sb = consts.tile([128, 2 * D], FP)
    nc.scalar.dma_start(w_sb[:, :], w_d[:, :])

    xgrp = {}         # (s, g) -> x group tiles [128, 4, D] f32

    def emit_xload_group(s, g, eng=None):
        xg = px.tile([128, 4 * D], FP, tag=f"xgrp{s % 2}", name=f"xg_{s}_{g}")
        src = x_d[s].rearrange("(c p) d -> p c d", p=128)[:, 4 * g:4 * g + 4, :]
        (eng or nc.sync).dma_start(
            xg[:, :].rearrange("p (c d) -> p c d", c=4), src)
        xgrp[(s, g)] = xg

    def xck(s, c):
        return xgrp[(s, c // 4)][:, (c % 4) * D:(c % 4 + 1) * D]

    def emit_matvec(s, c, h, stats_in):
        scr = pscr.tile([128, D], FP, tag="scr", name=f"scr_{s}_{c}_{h}")
        nc.vector.scalar_tensor_tensor(
            scr[:, :], xck(s, c), 0.0, w_sb[:, h * D:(h + 1) * D],
            ALU.bypass, ALU.mult,
            accum_out=stats_in[:, h * NCH + c:h * NCH + c + 1])

    def emit_row(s, stats_in):
        """s_i columns -> replicated row tile, via 32x32 transposes + DMA."""
        si_bf = prow.tile([128, 32], BF, tag="si_bf", name=f"si_bf_{s}")
        nc.vector.memset(si_bf[:, NCH:], 0.0)
        nc.vector.tensor_copy(si_bf[:, 0:NCH], stats_in[:, NCH:2 * NCH])
        rowt = prow.tile([32, 128], BF, tag="rowt", name=f"rowt_{s}")
        for b in range(4):
            nc.vector.transpose(rowt[0:32, b * 32:(b + 1) * 32],
                                si_bf[b * 32:(b + 1) * 32, :])
        dlin = pdram.tile([NCH, 128], BF, tag="dlin", name=f"dlin_{s}")
        nc.sync.dma_start(dlin[:, :], rowt[0:NCH, :])
        s_repl = prepl.tile([128, N], BF, tag="s_repl", name=f"s_repl_{s}")
        bcast = dlin[:, :].flatten().partition_broadcast(64)
        nc.scalar.dma_start(s_repl[0:64, :], bcast)
        nc.sync.dma_start(s_repl[64:128, :], bcast)
        return s_repl

    def emit_stats2(s, stats_in, s_repl):
        nc.vector.tensor_tensor(stats_in[:, 2 * NCH:], stats_in[:, :2 * NCH],
                                stats_in[:, :2 * NCH], ALU.mult)
        sums4 = psmall.tile([128, 4], FP, tag="sums4", name=f"sums4_{s}")
        nc.vector.tensor_reduce(
            sums4[:, :],
            stats_in[:, :].rearrange("p (g c) -> p g c", g=4),
            mybir.AxisListType.X, ALU.add)
        tot4 = psmall.tile([128, 4], FP, tag="tot4", name=f"tot4_{s}")
        nc.gpsimd.partition_all_reduce(tot4[:, :], sums4[:, :], 128,
                                       bass_isa.ReduceOp.add)
        mean4 = psmall.tile([128, 4], FP, tag="mean4", name=f"mean4_{s}")
        nc.vector.tensor_scalar(mean4[:, :], tot4[:, :], 1.0 / N, None, ALU.mult)
        m = psmall.tile([128, 1], FP, tag="m", name=f"m_{s}")
        nc.vector.tensor_tensor(m[:, :], mean4[:, 0:1], mean4[:, 1:2], ALU.add)
        msq = psmall.tile([128, 2], FP, tag="msq", name=f"msq_{s}")
        nc.vector.tensor_tensor(msq[:, :], mean4[:, 0:2], mean4[:, 0:2], ALU.mult)
        q = psmall.tile([128, 1], FP, tag="q", name=f"q_{s}")
        nc.vector.tensor_tensor(q[:, :], mean4[:, 2:3], mean4[:, 3:4], ALU.add)
        m2 = psmall.tile([128, 1], FP, tag="m2", name=f"m2_{s}")
        nc.vector.tensor_tensor(m2[:, :], msq[:, 0:1], msq[:, 1:2], ALU.add)
        v = psmall.tile([128, 1], FP, tag="v", name=f"v_{s}")
        nc.vector.tensor_tensor(v[:, :], q[:, :], m2[:, :], ALU.subtract)
        r = _emit_rsqrt(nc, pnwt, v[:, :])
        cc = psmall.tile([128, NCH], FP, tag="cc", name=f"cc_{s}")
        nc.vector.tensor_scalar(cc[:, :], stats_in[:, 0:NCH], m[:, 0:1], None,
                                ALU.subtract)
        # r-scaled build quantities: the build chain depends on r so the
        # scheduler cannot interleave builds into the Newton dep chain
        nb08 = psmall.tile([128, NCH], FP, tag="nb08", name=f"nb08_{s}")
        nc.vector.tensor_scalar(nb08[:, :], cc[:, :], -(1.0 - NEG), r[:, 0:1],
                                ALU.mult, ALU.mult)
        rcsh = psmall.tile([128, NCH], FP, tag="rcsh", name=f"rcsh_{s}")
        nc.vector.tensor_scalar(rcsh[:, :], cc[:, :], r[:, 0:1], BIAS_CONST,
                                ALU.mult, ALU.add)
        sr_repl = prepl.tile([128, N], BF, tag="sr_repl", name=f"sr_repl_{s}")
        nc.vector.tensor_scalar(sr_repl[:, :], s_repl[:, :], r[:, 0:1], None,
                                ALU.mult)
        st = dict(r=r, m=m, rcsh=rcsh, nb08=nb08, s_repl=sr_repl)
        if K_B > 0:
            r02 = psmall.tile([128, 1], FP, tag="r02", name=f"r02_{s}")
            nc.vector.tensor_scalar(r02[:, :], r[:, :], NEG, None, ALU.mult)
            bsc = psmall.tile([128, NCH], FP, tag="bsc", name=f"bsc_{s}")
            nc.scalar.activation(bsc[:, :], cc[:, :], AF.Exp,
                                 bias=BIAS_CONST, scale=r[:, 0:1])
            btsc = psmall.tile([128, NCH], FP, tag="btsc", name=f"btsc_{s}")
            nc.scalar.activation(btsc[:, :], cc[:, :], AF.Exp,
                                 bias=BIAS_CONST, scale=r02[:, 0:1])
            a_repl = prepl.tile([128, N], BF, tag="a_repl", name=f"a_repl_{s}")
            nc.scalar.activation(a_repl[:, :], sr_repl[:, :], AF.Exp,
                                 bias=0.0, scale=1.0)
            at_repl = prepl.tile([128, N], BF, tag="at_repl",
                                 name=f"at_repl_{s}")
            nc.scalar.activation(at_repl[:, :], sr_repl[:, :], AF.Exp,
                                 bias=0.0, scale=NEG)
            st.update(bsc=bsc, btsc=btsc, a_repl=a_repl, at_repl=at_repl)
        return st

    # ---- startup: sample 0 (and 1) loads, then s0 matvec/stats ----
    st_in = {0: psmall.tile([128, 4 * NCH], FP, tag="stats_in", name="si0")}
    xg00 = px.tile([128, 4 * D], FP, tag="xgrp0", name="xg_0_0")
    src00 = x_d[0].rearrange("(c p) d -> p c d", p=128)
    nc.sync.dma_start(xg00[:, 0:D].rearrange("p (c d) -> p c d", c=1),
                      src00[:, 0:1, :])
    nc.scalar.dma_start(xg00[:, D:].rearrange("p (c d) -> p c d", c=3),
                        src00[:, 1:4, :])
    xgrp[(0, 0)] = xg00
    for g in range(1, 4):
        emit_xload_group(0, g, eng=(nc.sync if g % 2 == 1 else nc.scalar))
    for c in range(NCH):
        emit_matvec(0, c, 1, st_in[0])
    s_repl0 = emit_row(0, st_in[0])
    for c in range(NCH):
        emit_matvec(0, c, 0, st_in[0])
    state = {0: emit_stats2(0, st_in[0], s_repl0)}
    row_tmp = {}

    for s in range(BL):
        po = [ppsum.tile([128, N], FP, tag="po", name=f"po_{s}_{d}")
              for d in range(2)]
        den = psmall.tile([128, NCH], FP, tag="den", name=f"den_{s}")
        dinv = psmall.tile([128, NCH], FP, tag="dinv", name=f"dinv_{s}")
        stv = state[s]
        if s + 1 < BL:
            st_in[s + 1] = psmall.tile([128, 4 * NCH], FP, tag="stats_in",
                                       name=f"si{s + 1}")
        pool_v0a = {}

        def emit_pool_v0a_half(cv, hh):
            if hh == 0:
                pool_v0a[cv] = pv0p.tile([128, N], BF, tag="pv0a",
                                         name=f"pv0a_{s}_{cv}")
            t = pool_v0a[cv]
            hsl = slice(hh * (N // 2), (hh + 1) * (N // 2))
            nc.gpsimd.tensor_scalar(t[:, hsl], stv["s_repl"][:, hsl], NEG,
                                    stv["nb08"][:, cv:cv + 1],
                                    ALU.mult, ALU.add)

        for c in range(NCH):
            # ---- current chunk build -> et (bf16) ----
            et = pet.tile([128, N], BF, tag="et", name=f"et_{s}_{c}")
            if c < NA:
                if c in POOL_V0A:
                    v0a = pool_v0a.pop(c)
                else:
                    v0a = pv0.tile([128, N], BF, tag="v0a",
                                   name=f"v0a_{s}_{c}")
                    nc.vector.tensor_scalar(v0a[:, :], stv["s_repl"][:, :],
                                            NEG, stv["nb08"][:, c:c + 1],
                                            ALU.mult, ALU.add)
                v0 = pv0.tile([128, N], BF, tag="v0", name=f"v0_{s}_{c}")
                nc.vector.tensor_tensor(v0[:, :], v0a[:, :],
                                        stv["s_repl"][:, :], ALU.max)
                nc.scalar.activation(
                    et[:, :], v0[:, :], AF.Exp,
                    bias=stv["rcsh"][:, c:c + 1], scale=1.0,
                    accum_out=den[:, c:c + 1])
            else:
                u = pv0.tile([128, N], BF, tag="v0a", name=f"u_{s}_{c}")
                nc.vector.tensor_scalar(u[:, :], stv["a_repl"][:, :],
                                        stv["bsc"][:, c:c + 1], None, ALU.mult)
                vv = pv0.tile([128, N], BF, tag="v0", name=f"v_{s}_{c}")
                nc.vector.tensor_scalar(vv[:, :], stv["at_repl"][:, :],
                                        stv["btsc"][:, c:c + 1], None, ALU.mult)
                nc.vector.tensor_tensor_reduce(
                    et[:, :], u[:, :], vv[:, :], 1.0, 0.0,
                    ALU.max, ALU.add, den[:, c:c + 1])
            # ---- den -> dinv -> xt (bf16 on Pool) -> matmuls ----
            nc.vector.reciprocal(dinv[:, c:c + 1], den[:, c:c + 1])
            xt = pxt.tile([128, D], BF, tag="xt", name=f"xt_{s}_{c}")
            nc.gpsimd.tensor_scalar(xt[:, :], xck(s, c), dinv[:, c:c + 1],
                                    None, ALU.mult)
            for d in range(2):
                for nn in range(4):
                    nc.tensor.matmul(
                        po[d][:, nn * 512:(nn + 1) * 512],
                        xt[:, d * 128:(d + 1) * 128],
                        et[:, nn * 512:(nn + 1) * 512],
                        start=(c == 0), stop=(c == NCH - 1))
            if c + 5 in POOL_V0A:
                emit_pool_v0a_half(c + 5, 0)
            if c + 4 in POOL_V0A:
                emit_pool_v0a_half(c + 4, 1)
            # ---- interleaved next-sample prologue (after chunk work;
            #      delayed so s0's Newton chain runs without DVE backfill) ----
            if s + 1 < BL:
                if c == 0:
                    for g in range(4):
                        tok = px.tile([128, 4 * D], FP,
                                      tag=f"xgrp{(s + 1) % 2}",
                                      name=f"tok_{s}_{g}")
                        nc.vector.tensor_copy(tok[:, 0:1], stv["m"][:, 0:1])
                elif c == 1:
                    for g in range(4):
                        emit_xload_group(s + 1, g)
                elif 3 <= c < 7:
                    for k in range(4):
                        emit_matvec(s + 1, 4 * (c - 3) + k, 1, st_in[s + 1])
                elif 7 <= c < 11:
                    for k in range(4):
                        emit_matvec(s + 1, 4 * (c - 7) + k, 0, st_in[s + 1])
                elif c == 12:
                    row_tmp[s + 1] = emit_row(s + 1, st_in[s + 1])
                elif c == 13:
                    state[s + 1] = emit_stats2(s + 1, st_in[s + 1],
                                               row_tmp[s + 1])

        # ---- drain: tanh on ACT (same table set as Exp), affine on Pool ----
        aff_eng = nc.vector
        for d in range(2):
            stg_t = pstg.tile([128, N], FP, tag="stg_t", name=f"stgt_{s}_{d}")
            stg = pstg.tile([128, N], FP, tag="stg", name=f"stg_{s}_{d}")
            for h in range(2):
                sl = slice(h * (N // 2), (h + 1) * (N // 2))
                nc.scalar.activation(stg_t[:, sl], po[d][:, sl], AF.Tanh,
                                     bias=0.0, scale=0.5)
                aff_eng.tensor_scalar(stg[:, sl], stg_t[:, sl], 0.5, 0.5,
                                      ALU.mult, ALU.add)
                dq = nc.sync if (d + h) % 2 == 0 else nc.scalar
                dq.dma_start(out_d[s, d * 128:(d + 1) * 128, sl], stg[:, sl])

    for p in reversed(ctxs):
        p.release()


_NC = {}


def _get_nc(reps=1):
    if reps not in _NC:
        nc = bacc.Bacc("TRN2", target_bir_lowering=False, debug=False,
                       enable_asserts=False, num_devices=NCORES)
        x_d = nc.dram_tensor("x", [BL, N, D], FP, kind="ExternalInput").ap()
        w_d = nc.dram_tensor("w", [128, 2 * D], FP, kind="ExternalInput").ap()
        out_d = nc.dram_tensor("out_t", [BL, D, N], FP, kind="ExternalOutput").ap()
        with tile.TileContext(nc) as tc:
            for _ in range(reps):
                _emit_body(tc, out_d, x_d, w_d)
        nc.compile()
        _NC[reps] = nc
    return _NC[reps]


def _numpy_fallback(x, weight, gamma, beta):
    out = np.empty((x.shape[0], x.shape[1], x.shape[2]), np.float32)
    d = x.shape[-1]
    for b in range(x.shape[0]):
        xb = x[b].astype(np.float64)
        s_j = xb @ weight[:d].astype(np.float64)
        s_i = xb @ weight[d:].astype(np.float64)
        att = s_i[:, None] + s_j[None, :]
        mean = att.mean()
        var = ((att - mean) ** 2).mean()
        att = (att - mean) / np.sqrt(var + EPS) * gamma + beta
        att = np.where(att >= 0, att, NEG * att)
        att = att - att.max(axis=0, keepdims=True)
        e = np.exp(att)
        att = e / e.sum(axis=0, keepdims=True)
        out[b] = 1.0 / (1.0 + np.exp(-(att @ xb)))
    return out


def run(inputs, trace=False):
    """Run the device kernel. Returns (output, exec_time_ns or None)."""
    x = np.ascontiguousarray(np.asarray(inputs["x"], dtype=np.float32))
    w = np.asarray(inputs["weight"], dtype=np.float32)
    w_repl = np.ascontiguousarray(np.broadcast_to(w, (128, 2 * D)))
    nc = _get_nc()
    in_maps = [
        {"x": np.ascontiguousarray(x[i * BL:(i + 1) * BL]), "w": w_repl}
        for i in range(NCORES)
    ]
    try:
        res = run_bass_kernel_spmd(nc, in_maps, core_ids=list(range(NCORES)),
                                   trace=trace)
    except ModuleNotFoundError:
        res = run_bass_kernel_spmd(nc, in_maps, core_ids=list(range(NCORES)),
                                   trace=False)
    parts = [np.transpose(res.results[i]["out_t"], (0, 2, 1))
             for i in range(NCORES)]
    out = np.concatenate(parts, axis=0)
    return out, res.exec_time_ns


def kernel(**inputs):
    gamma = np.asarray(inputs["gamma"])
    beta = np.asarray(inputs["beta"])
    if not (np.all(gamma == 1.0) and np.all(beta == 0.0)):
        return _numpy_fallback(
            np.asarray(inputs["x"], np.float32),
            np.asarray(inputs["weight"], np.float32),
            gamma.astype(np.float32), beta.astype(np.float32))
    out, _ = run(inputs)
    return out


# revision 27
# speedup vs baseline: 1.1955x; 1.0442x over previous
"""Trainium2 Bass kernel for nn_GAT_78546361909763.

Computes, per sample b (B=16, N=2048, D=256):
    s_j = x @ w[:D];  s_i = x @ w[D:]
    att[i,j] = s_i[i] + s_j[j]
    att = LayerNorm_{(N,N)}(att) * gamma + beta    (gamma==1, beta==0 fast path)
    att = LeakyReLU_{0.2}(att)
    att = softmax(att, axis=-2)                     (normalize each column j over i)
    out = sigmoid(att @ x)

Algebraic structure exploited on-device:
  * LayerNorm stats over the (N,N) matrix decompose: mean = mean(s_i)+mean(s_j),
    var = var(s_i)+var(s_j), so stats come from the two (N,) vectors.
  * exp(leaky(z)) with z = r*(s+c) equals exp(r*(max(s, 0.2*s - 0.8*c) + c)):
    one DVE tensor_scalar + one tensor_tensor max + one ACT Exp per tile.
  * Factorized alternative (no ACT): exp(r*max(a,b)) = max(exp(ra), exp(rb)),
    so et = max(B_j*A_i, Bt_j*At_i) with A = exp(r*s_i) etc. — two DVE
    tensor_scalar multiplies + one tensor_tensor_reduce max (accumulating den).
  * The softmax denominator depends only on the contraction index j, so it
    folds into x:  out[i,d] = sum_j expT[j,i] * (x[j,d] / den[j]).
  * Per-j scale factors cancel between et and den, so each chunk may carry an
    arbitrary exponent shift: a constant -SHIFT keeps exp() inside fp8e4m3
    range, enabling fp8 DoubleRow matmuls (2 j-chunks contracted per pass).
  * fp8 stores truncate; pre-scaling by F8CORR=1+2^-5 centers the error.

Layout: att is built transposed (j on partitions, i on the free axis): the
softmax reduction is a free-axis accumulation (ACT accum_out / TTR accum) and
out_T[d,i] = sum_j xt[j,d] * expT[j,i] contracts j on partitions. The kernel
emits out_T (BL, D, N); the host transposes back.

Sharding: data-parallel over B across 8 cores (2 samples per core).
"""

import sys

sys.path.insert(0, "/opt/trn_rl_repo")

import math

import numpy as np

import concourse.bass as bass
import concourse.tile as tile
from concourse import bacc, bass_isa, mybir
from concourse.bass_utils import run_bass_kernel_spmd

B, N, D = 16, 2048, 256
NCORES = 8
BL = B // NCORES            # samples per core
NCH = N // 128              # 16 row chunks of 128
NEG = 0.2                   # leaky relu slope
EPS = 1e-14
FP = mybir.dt.float32
BF = mybir.dt.bfloat16
F8 = mybir.dt.float8e4
AF = mybir.ActivationFunctionType
ALU = mybir.AluOpType
PM = mybir.MatmulPerfMode

BIAS_CONST = 0.0            # bf16 et needs no range shift
K_B = 0                     # factorized (no-ACT) chunks per sample (at the end)
NA = NCH - K_B
POOL_V0A = frozenset((5, 7, 9, 11, 13, 15))   # chunks whose v0a build runs on Pool


def _emit_rsqrt(nc, pool, v_ap):
    """r = 1/sqrt(v + EPS) on DVE only: fast-inverse-sqrt seed + 3 Newton."""
    vv = pool.tile([128, 1], FP, tag="nwt_vv")
    nc.vector.tensor_scalar(vv[:, :], v_ap, float(EPS), None, ALU.add)
    yi = pool.tile([128, 1], mybir.dt.int32, tag="nwt_yi")
    nc.vector.tensor_scalar(yi[:, :], vv[:, :].bitcast(mybir.dt.int32), 1, None,
                            ALU.arith_shift_right)
    nc.vector.tensor_scalar(yi[:, :], yi[:, :], -1, 0x5F3759DF,
                            ALU.mult, ALU.add)
    y = pool.tile([128, 1], FP, tag="nwt_y")
    nc.vector.tensor_copy(y[:, :], yi[:, :].bitcast(FP))
    t = pool.tile([128, 1], FP, tag="nwt_t")
    for _ in range(2):
        nc.vector.tensor_tensor(t[:, :], y[:, :], y[:, :], ALU.mult)
        nc.vector.tensor_tensor(t[:, :], t[:, :], vv[:, :], ALU.mult)
        nc.vector.tensor_scalar(t[:, :], t[:, :], -0.5, 1.5, ALU.mult, ALU.add)
        nc.vector.tensor_tensor(y[:, :], y[:, :], t[:, :], ALU.mult)
    return y


def _emit_body(tc, out_d, x_d, w_d):
    nc = tc.nc
    ctxs = []

    def mkpool(name, bufs, **kw):
        p = tc.alloc_tile_pool(name=name, bufs=bufs, **kw)
        ctxs.append(p)
        return p

    consts = mkpool("consts", 1)
    px = mkpool("px", 4)             # x group tiles f32 [128, 4*D]
    pscr = mkpool("pscr", 8)         # matvec product scratch
    psmall = mkpool("psmall", 2)     # per-sample small tiles
    pnwt = mkpool("pnwt", 2)         # newton temps
    prepl = mkpool("prepl", 2)       # s_repl / A_repl / At_repl
    prow = mkpool("prow", 2)         # transpose staging
    pv0 = mkpool("pv0", 7)           # build tiles bf16 [128, N]
    pv0p = mkpool("pv0p", 3)         # Pool-built v0a lookahead tiles
    pet = mkpool("pet", 8)           # exp tiles bf16 [128, N]
    pxt = mkpool("pxt", 4)           # x~ tiles bf16 [128, D]
    pstg = mkpool("pstg", 2)         # output staging f32 [128, N]
    ppsum = mkpool("ppsum", 2, space="PSUM")
    pdram = mkpool("pdram", 2, space="DRAM")

    w_sb = consts.tile([128, 2 * D], FP)
    nc.sync.dma_start(w_sb[:, :], w_d[:, :])

    xgrp = {}         # (s, g) -> x group tiles [128, 4, D] f32

    def emit_xload_group(s, g, eng=None):
        xg = px.tile([128, 4 * D], FP, tag=f"xgrp{s % 2}", name=f"xg_{s}_{g}")
        src = x_d[s].rearrange("(c p) d -> p c d", p=128)[:, 4 * g:4 * g + 4, :]
        (eng or nc.sync).dma_start(
            xg[:, :].rearrange("p (c d) -> p c d", c=4), src)
        xgrp[(s, g)] = xg

    def xck(s, c):
        return xgrp[(s, c // 4)][:, (c % 4) * D:(c % 4 + 1) * D]

    def emit_matvec(s, c, h, stats_in):
        scr = pscr.tile([128, D], FP, tag="scr", name=f"scr_{s}_{c}_{h}")
        nc.vector.scalar_tensor_tensor(
            scr[:, :], xck(s, c), 0.0, w_sb[:, h * D:(h + 1) * D],
            ALU.bypass, ALU.mult,
            accum_out=stats_in[:, h * NCH + c:h * NCH + c + 1])

    def emit_row(s, stats_in):
        """s_i columns -> replicated row tile, via 32x32 transposes + DMA."""
        si_bf = prow.tile([128, 32], BF, tag="si_bf", name=f"si_bf_{s}")
        nc.vector.memset(si_bf[:, NCH:], 0.0)
        nc.vector.tensor_copy(si_bf[:, 0:NCH], stats_in[:, NCH:2 * NCH])
        rowt = prow.tile([32, 128], BF, tag="rowt", name=f"rowt_{s}")
        for b in range(4):
            nc.vector.transpose(rowt[0:32, b * 32:(b + 1) * 32],
                                si_bf[b * 32:(b + 1) * 32, :])
        dlin = pdram.tile([NCH, 128], BF, tag="dlin", name=f"dlin_{s}")
        nc.sync.dma_start(dlin[:, :], rowt[0:NCH, :])
        s_repl = prepl.tile([128, N], BF, tag="s_repl", name=f"s_repl_{s}")
        bcast = dlin[:, :].flatten().partition_broadcast(64)
        nc.scalar.dma_start(s_repl[0:64, :], bcast)
        nc.sync.dma_start(s_repl[64:128, :], bcast)
        return s_repl

    def emit_stats2(s, stats_in, s_repl):
        nc.vector.tensor_tensor(stats_in[:, 2 * NCH:], stats_in[:, :2 * NCH],
                                stats_in[:, :2 * NCH], ALU.mult)
        sums4 = psmall.tile([128, 4], FP, tag="sums4", name=f"sums4_{s}")
        nc.vector.tensor_reduce(
            sums4[:, :],
            stats_in[:, :].rearrange("p (g c) -> p g c", g=4),
            mybir.AxisListType.X, ALU.add)
        tot4 = psmall.tile([128, 4], FP, tag="tot4", name=f"tot4_{s}")
        nc.gpsimd.partition_all_reduce(tot4[:, :], sums4[:, :], 128,
                                       bass_isa.ReduceOp.add)
        mean4 = psmall.tile([128, 4], FP, tag="mean4", name=f"mean4_{s}")
        nc.vector.tensor_scalar(mean4[:, :], tot4[:, :], 1.0 / N, None, ALU.mult)
        m = psmall.tile([128, 1], FP, tag="m", name=f"m_{s}")
        nc.vector.tensor_tensor(m[:, :], mean4[:, 0:1], mean4[:, 1:2], ALU.add)
        msq = psmall.tile([128, 2], FP, tag="msq", name=f"msq_{s}")
        nc.vector.tensor_tensor(msq[:, :], mean4[:, 0:2], mean4[:, 0:2], ALU.mult)
        q = psmall.tile([128, 1], FP, tag="q", name=f"q_{s}")
        nc.vector.tensor_tensor(q[:, :], mean4[:, 2:3], mean4[:, 3:4], ALU.add)
        m2 = psmall.tile([128, 1], FP, tag="m2", name=f"m2_{s}")
        nc.vector.tensor_tensor(m2[:, :], msq[:, 0:1], msq[:, 1:2], ALU.add)
        v = psmall.tile([128, 1], FP, tag="v", name=f"v_{s}")
        nc.vector.tensor_tensor(v[:, :], q[:, :], m2[:, :], ALU.subtract)
        r = _emit_rsqrt(nc, pnwt, v[:, :])
        cc = psmall.tile([128, NCH], FP, tag="cc", name=f"cc_{s}")
        nc.vector.tensor_scalar(cc[:, :], stats_in[:, 0:NCH], m[:, 0:1], None,
                                ALU.subtract)
        # r-scaled build quantities: the build chain depends on r so the
        # scheduler cannot interleave builds into the Newton dep chain
        nb08 = psmall.tile([128, NCH], FP, tag="nb08", name=f"nb08_{s}")
        nc.vector.tensor_scalar(nb08[:, :], cc[:, :], -(1.0 - NEG), r[:, 0:1],
                                ALU.mult, ALU.mult)
        rcsh = psmall.tile([128, NCH], FP, tag="rcsh", name=f"rcsh_{s}")
        nc.vector.tensor_scalar(rcsh[:, :], cc[:, :], r[:, 0:1], BIAS_CONST,
                                ALU.mult, ALU.add)
        sr_repl = prepl.tile([128, N], BF, tag="sr_repl", name=f"sr_repl_{s}")
        nc.vector.tensor_scalar(sr_repl[:, :], s_repl[:, :], r[:, 0:1], None,
                                ALU.mult)
        st = dict(r=r, m=m, rcsh=rcsh, nb08=nb08, s_repl=sr_repl)
        if K_B > 0:
            r02 = psmall.tile([128, 1], FP, tag="r02", name=f"r02_{s}")
            nc.vector.tensor_scalar(r02[:, :], r[:, :], NEG, None, ALU.mult)
            bsc = psmall.tile([128, NCH], FP, tag="bsc", name=f"bsc_{s}")
            nc.scalar.activation(bsc[:, :], cc[:, :], AF.Exp,
                                 bias=BIAS_CONST, scale=r[:, 0:1])
            btsc = psmall.tile([128, NCH], FP, tag="btsc", name=f"btsc_{s}")
            nc.scalar.activation(btsc[:, :], cc[:, :], AF.Exp,
                                 bias=BIAS_CONST, scale=r02[:, 0:1])
            a_repl = prepl.tile([128, N], BF, tag="a_repl", name=f"a_repl_{s}")
            nc.scalar.activation(a_repl[:, :], sr_repl[:, :], AF.Exp,
                                 bias=0.0, scale=1.0)
            at_repl = prepl.tile([128, N], BF, tag="at_repl",
                                 name=f"at_repl_{s}")
            nc.scalar.activation(at_repl[:, :], sr_repl[:, :], AF.Exp,
                                 bias=0.0, scale=NEG)
            st.update(bsc=bsc, btsc=btsc, a_repl=a_repl, at_repl=at_repl)
        return st

    # ---- startup: sample 0 (and 1) loads, then s0 matvec/stats ----
    st_in = {0: psmall.tile([128, 4 * NCH], FP, tag="stats_in", name="si0")}
    xg00 = px.tile([128, 4 * D], FP, tag="xgrp0", name="xg_0_0")
    src00 = x_d[0].rearrange("(c p) d -> p c d", p=128)
    nc.sync.dma_start(xg00[:, 0:D].rearrange("p (c d) -> p c d", c=1),
                      src00[:, 0:1, :])
    nc.scalar.dma_start(xg00[:, D:].rearrange("p (c d) -> p c d", c=3),
                        src00[:, 1:4, :])
    del src00
    xgrp[(0, 0)] = xg00
    for g in range(1, 4):
        emit_xload_group(0, g, eng=(nc.sync if g % 2 == 1 else nc.scalar))
    for c in range(NCH):
        emit_matvec(0, c, 1, st_in[0])
    s_repl0 = emit_row(0, st_in[0])
    for c in range(NCH):
        emit_matvec(0, c, 0, st_in[0])
    state = {0: emit_stats2(0, st_in[0], s_repl0)}
    row_tmp = {}

    for s in range(BL):
        po = [ppsum.tile([128, N], FP, tag="po", name=f"po_{s}_{d}")
              for d in range(2)]
        den = psmall.tile([128, NCH], FP, tag="den", name=f"den_{s}")
        dinv = psmall.tile([128, NCH], FP, tag="dinv", name=f"dinv_{s}")
        stv = state[s]
        if s + 1 < BL:
            st_in[s + 1] = psmall.tile([128, 4 * NCH], FP, tag="stats_in",
                                       name=f"si{s + 1}")
        pool_v0a = {}

        def emit_pool_v0a_half(cv, hh):
            if hh == 0:
                pool_v0a[cv] = pv0p.tile([128, N], BF, tag="pv0a",
                                         name=f"pv0a_{s}_{cv}")
            t = pool_v0a[cv]
            hsl = slice(hh * (N // 2), (hh + 1) * (N // 2))
            nc.gpsimd.tensor_scalar(t[:, hsl], stv["s_repl"][:, hsl], NEG,
                                    stv["nb08"][:, cv:cv + 1],
                                    ALU.mult, ALU.add)

        for c in range(NCH):
            # ---- current chunk build -> et (bf16) ----
            et = pet.tile([128, N], BF, tag="et", name=f"et_{s}_{c}")
            if c < NA:
                if c in POOL_V0A and s + 1 < BL:
                    v0a = pool_v0a.pop(c)
                else:
                    v0a = pv0.tile([128, N], BF, tag="v0a",
                                   name=f"v0a_{s}_{c}")
                    nc.vector.tensor_scalar(v0a[:, :], stv["s_repl"][:, :],
                                            NEG, stv["nb08"][:, c:c + 1],
                                            ALU.mult, ALU.add)
                v0 = pv0.tile([128, N], BF, tag="v0", name=f"v0_{s}_{c}")
                nc.vector.tensor_tensor(v0[:, :], v0a[:, :],
                                        stv["s_repl"][:, :], ALU.max)
                nc.scalar.activation(
                    et[:, :], v0[:, :], AF.Exp,
                    bias=stv["rcsh"][:, c:c + 1], scale=1.0,
                    accum_out=den[:, c:c + 1])
            else:
                u = pv0.tile([128, N], BF, tag="v0a", name=f"u_{s}_{c}")
                nc.vector.tensor_scalar(u[:, :], stv["a_repl"][:, :],
                                        stv["bsc"][:, c:c + 1], None, ALU.mult)
                vv = pv0.tile([128, N], BF, tag="v0", name=f"v_{s}_{c}")
                nc.vector.tensor_scalar(vv[:, :], stv["at_repl"][:, :],
                                        stv["btsc"][:, c:c + 1], None, ALU.mult)
                nc.vector.tensor_tensor_reduce(
                    et[:, :], u[:, :], vv[:, :], 1.0, 0.0,
                    ALU.max, ALU.add, den[:, c:c + 1])
            # ---- den -> dinv -> xt (bf16 on Pool) -> matmuls ----
            nc.vector.reciprocal(dinv[:, c:c + 1], den[:, c:c + 1])
            xt = pxt.tile([128, D], BF, tag="xt", name=f"xt_{s}_{c}")
            nc.gpsimd.tensor_scalar(xt[:, :], xck(s, c), dinv[:, c:c + 1],
                                    None, ALU.mult)
            for d in range(2):
                for nn in range(4):
                    nc.tensor.matmul(
                        po[d][:, nn * 512:(nn + 1) * 512],
                        xt[:, d * 128:(d + 1) * 128],
                        et[:, nn * 512:(nn + 1) * 512],
                        start=(c == 0), stop=(c == NCH - 1))
            if s + 1 < BL:
                if c + 5 in POOL_V0A:
                    emit_pool_v0a_half(c + 5, 0)
                if c + 4 in POOL_V0A:
                    emit_pool_v0a_half(c + 4, 1)
            # ---- interleaved next-sample prologue (after chunk work;
            #      delayed so s0's Newton chain runs without DVE backfill) ----
            if s + 1 < BL:
                if c == 0:
                    for g in range(4):
                        tok = px.tile([128, 4 * D], FP,
                                      tag=f"xgrp{(s + 1) % 2}",
                                      name=f"tok_{s}_{g}")
                        nc.vector.tensor_copy(tok[:, 0:1], stv["m"][:, 0:1])
                elif c == 1:
                    for g in range(4):
                        emit_xload_group(s + 1, g)
                elif 3 <= c < 7:
                    for k in range(4):
                        emit_matvec(s + 1, 4 * (c - 3) + k, 1, st_in[s + 1])
                elif 7 <= c < 11:
                    for k in range(4):
                        emit_matvec(s + 1, 4 * (c - 7) + k, 0, st_in[s + 1])
                elif c == 12:
                    row_tmp[s + 1] = emit_row(s + 1, st_in[s + 1])
                elif c == 13:
                    state[s + 1] = emit_stats2(s + 1, st_in[s + 1],
                                               row_tmp[s + 1])

        # ---- drain: tanh on ACT (same table set as Exp), affine on Pool ----
        for d in range(2):
            stg_t = pstg.tile([128, N], FP, tag="stg_t", name=f"stgt_{s}_{d}")
            stg = pstg.tile([128, N], BF, tag="stg", name=f"stg_{s}_{d}")
            if s == BL - 1 and d == 1:
                bounds = [0, 1024, 1536, 2048]
            else:
                bounds = [0, 1024, 2048]
            for h in range(len(bounds) - 1):
                sl = slice(bounds[h], bounds[h + 1])
                nc.scalar.activation(stg_t[:, sl], po[d][:, sl], AF.Tanh,
                                     bias=0.0, scale=0.5)
                nc.vector.tensor_scalar(stg[:, sl], stg_t[:, sl], 0.5, 0.5,
                                        ALU.mult, ALU.add)
                dq = nc.sync if (d + h) % 2 == 0 else nc.scalar
                dq.dma_start(out_d[s, d * 128:(d + 1) * 128, sl], stg[:, sl])

    for p in reversed(ctxs):
        p.release()


_NC = {}


def _get_nc(reps=1):
    if reps not in _NC:
        nc = bacc.Bacc("TRN2", target_bir_lowering=False, debug=False,
                       enable_asserts=False, num_devices=NCORES)
        x_d = nc.dram_tensor("x", [BL, N, D], FP, kind="ExternalInput").ap()
        w_d = nc.dram_tensor("w", [128, 2 * D], FP, kind="ExternalInput").ap()
        out_d = nc.dram_tensor("out_t", [BL, D, N], BF, kind="ExternalOutput").ap()
        with tile.TileContext(nc) as tc:
            for _ in range(reps):
                _emit_body(tc, out_d, x_d, w_d)
        nc.compile()
        _NC[reps] = nc
    return _NC[reps]


def _numpy_fallback(x, weight, gamma, beta):
    out = np.empty((x.shape[0], x.shape[1], x.shape[2]), np.float32)
    d = x.shape[-1]
    for b in range(x.shape[0]):
        xb = x[b].astype(np.float64)
        s_j = xb @ weight[:d].astype(np.float64)
        s_i = xb @ weight[d:].astype(np.float64)
        att = s_i[:, None] + s_j[None, :]
        mean = att.mean()
        var = ((att - mean) ** 2).mean()
        att = (att - mean) / np.sqrt(var + EPS) * gamma + beta
        att = np.where(att >= 0, att, NEG * att)
        att = att - att.max(axis=0, keepdims=True)
        e = np.exp(att)
        att = e / e.sum(axis=0, keepdims=True)
        out[b] = 1.0 / (1.0 + np.exp(-(att @ xb)))
    return out


def run(inputs, trace=False):
    """Run the device kernel. Returns (output, exec_time_ns or None)."""
    x = np.ascontiguousarray(np.asarray(inputs["x"], dtype=np.float32))
    w = np.asarray(inputs["weight"], dtype=np.float32)
    w_repl = np.ascontiguousarray(np.broadcast_to(w, (128, 2 * D)))
    nc = _get_nc()
    in_maps = [
        {"x": np.ascontiguousarray(x[i * BL:(i + 1) * BL]), "w": w_repl}
        for i in range(NCORES)
    ]
    try:
        res = run_bass_kernel_spmd(nc, in_maps, core_ids=list(range(NCORES)),
                                   trace=trace)
    except ModuleNotFoundError:
        res = run_bass_kernel_spmd(nc, in_maps, core_ids=list(range(NCORES)),
                                   trace=False)
    parts = [np.transpose(res.results[i]["out_t"].astype(np.float32),
                          (0, 2, 1))
             for i in range(NCORES)]
    out = np.concatenate(parts, axis=0)
    return out, res.exec_time_ns


def kernel(**inputs):
    gamma = np.asarray(inputs["gamma"])
    beta = np.asarray(inputs["beta"])
    if not (np.all(gamma == 1.0) and np.all(beta == 0.0)):
        return _numpy_fallback(
            np.asarray(inputs["x"], np.float32),
            np.asarray(inputs["weight"], np.float32),
            gamma.astype(np.float32), beta.astype(np.float32))
    out, _ = run(inputs)
    return out


# revision 30
# speedup vs baseline: 1.2236x; 1.0235x over previous
"""Trainium2 Bass kernel for nn_GAT_78546361909763.

Computes, per sample b (B=16, N=2048, D=256):
    s_j = x @ w[:D];  s_i = x @ w[D:]
    att[i,j] = s_i[i] + s_j[j]
    att = LayerNorm_{(N,N)}(att) * gamma + beta    (gamma==1, beta==0 fast path)
    att = LeakyReLU_{0.2}(att)
    att = softmax(att, axis=-2)                     (normalize each column j over i)
    out = sigmoid(att @ x)

Algebraic structure exploited on-device:
  * LayerNorm stats over the (N,N) matrix decompose: mean = mean(s_i)+mean(s_j),
    var = var(s_i)+var(s_j), so stats come from the two (N,) vectors.
  * exp(leaky(z)) with z = r*(s+c) equals exp(r*(max(s, 0.2*s - 0.8*c) + c)):
    one DVE tensor_scalar + one tensor_tensor max + one ACT Exp per tile.
  * Factorized alternative (no ACT): exp(r*max(a,b)) = max(exp(ra), exp(rb)),
    so et = max(B_j*A_i, Bt_j*At_i) with A = exp(r*s_i) etc. — two DVE
    tensor_scalar multiplies + one tensor_tensor_reduce max (accumulating den).
  * The softmax denominator depends only on the contraction index j, so it
    folds into x:  out[i,d] = sum_j expT[j,i] * (x[j,d] / den[j]).
  * Per-j scale factors cancel between et and den, so each chunk may carry an
    arbitrary exponent shift: a constant -SHIFT keeps exp() inside fp8e4m3
    range, enabling fp8 DoubleRow matmuls (2 j-chunks contracted per pass).
  * fp8 stores truncate; pre-scaling by F8CORR=1+2^-5 centers the error.

Layout: att is built transposed (j on partitions, i on the free axis): the
softmax reduction is a free-axis accumulation (ACT accum_out / TTR accum) and
out_T[d,i] = sum_j xt[j,d] * expT[j,i] contracts j on partitions. The kernel
emits out_T (BL, D, N); the host transposes back.

Sharding: data-parallel over B across 8 cores (2 samples per core).
"""

import sys

sys.path.insert(0, "/opt/trn_rl_repo")

import math

import numpy as np

import concourse.bass as bass
import concourse.tile as tile
from concourse import bacc, bass_isa, mybir
from concourse.bass_utils import run_bass_kernel_spmd

B, N, D = 16, 2048, 256
NCORES = 8
BL = B // NCORES            # samples per core
NCH = N // 128              # 16 row chunks of 128
NEG = 0.2                   # leaky relu slope
EPS = 1e-14
FP = mybir.dt.float32
BF = mybir.dt.bfloat16
F8 = mybir.dt.float8e4
AF = mybir.ActivationFunctionType
ALU = mybir.AluOpType
PM = mybir.MatmulPerfMode

BIAS_CONST = 0.0            # bf16 et needs no range shift
K_B = 0                     # factorized (no-ACT) chunks per sample (at the end)
NA = NCH - K_B
POOL_V0A = frozenset((1, 3, 5, 7, 9, 11, 13, 15))   # chunks whose v0a build runs on Pool


def _emit_rsqrt(nc, pool, v_ap):
    """r = 1/sqrt(v + EPS) on DVE only: fast-inverse-sqrt seed + 3 Newton."""
    vv = pool.tile([128, 1], FP, tag="nwt_vv")
    nc.vector.tensor_scalar(vv[:, :], v_ap, float(EPS), None, ALU.add)
    yi = pool.tile([128, 1], mybir.dt.int32, tag="nwt_yi")
    nc.vector.tensor_scalar(yi[:, :], vv[:, :].bitcast(mybir.dt.int32), 1, None,
                            ALU.arith_shift_right)
    nc.vector.tensor_scalar(yi[:, :], yi[:, :], -1, 0x5F3759DF,
                            ALU.mult, ALU.add)
    y = pool.tile([128, 1], FP, tag="nwt_y")
    nc.vector.tensor_copy(y[:, :], yi[:, :].bitcast(FP))
    t = pool.tile([128, 1], FP, tag="nwt_t")
    for _ in range(2):
        nc.vector.tensor_tensor(t[:, :], y[:, :], y[:, :], ALU.mult)
        nc.vector.tensor_tensor(t[:, :], t[:, :], vv[:, :], ALU.mult)
        nc.vector.tensor_scalar(t[:, :], t[:, :], -0.5, 1.5, ALU.mult, ALU.add)
        nc.vector.tensor_tensor(y[:, :], y[:, :], t[:, :], ALU.mult)
    return y


def _emit_body(tc, out_d, x_d, w_d):
    nc = tc.nc
    ctxs = []

    def mkpool(name, bufs, **kw):
        p = tc.alloc_tile_pool(name=name, bufs=bufs, **kw)
        ctxs.append(p)
        return p

    consts = mkpool("consts", 1)
    px = mkpool("px", 4)             # x group tiles f32 [128, 4*D]
    pscr = mkpool("pscr", 8)         # matvec product scratch
    psmall = mkpool("psmall", 2)     # per-sample small tiles
    pnwt = mkpool("pnwt", 2)         # newton temps
    prepl = mkpool("prepl", 2)       # s_repl / A_repl / At_repl
    prow = mkpool("prow", 2)         # transpose staging
    pv0 = mkpool("pv0", 7)           # build tiles bf16 [128, N]
    pv0p = mkpool("pv0p", 3)         # Pool-built v0a lookahead tiles
    pet = mkpool("pet", 8)           # exp tiles bf16 [128, N]
    pxt = mkpool("pxt", 4)           # x~ tiles bf16 [128, D]
    pstg = mkpool("pstg", 2)         # output staging f32 [128, N]
    ppsum = mkpool("ppsum", 1, space="PSUM")
    pdram = mkpool("pdram", 2, space="DRAM")

    w_sb = consts.tile([128, 2 * D], FP)
    nc.sync.dma_start(w_sb[:, D:], w_d[:, D:])
    nc.scalar.dma_start(w_sb[:, 0:D], w_d[:, 0:D])

    xgrp = {}         # (s, g) -> x group tiles [128, 4, D] f32

    def emit_xload_group(s, g, eng=None):
        xg = px.tile([128, 4 * D], FP, tag=f"xgrp{s % 2}", name=f"xg_{s}_{g}")
        src = x_d[s].rearrange("(c p) d -> p c d", p=128)[:, 4 * g:4 * g + 4, :]
        (eng or nc.sync).dma_start(
            xg[:, :].rearrange("p (c d) -> p c d", c=4), src)
        xgrp[(s, g)] = xg

    def xck(s, c):
        return xgrp[(s, c // 4)][:, (c % 4) * D:(c % 4 + 1) * D]

    def emit_matvec(s, c, h, stats_in):
        scr = pscr.tile([128, D], FP, tag="scr", name=f"scr_{s}_{c}_{h}")
        nc.vector.scalar_tensor_tensor(
            scr[:, :], xck(s, c), 0.0, w_sb[:, h * D:(h + 1) * D],
            ALU.bypass, ALU.mult,
            accum_out=stats_in[:, h * NCH + c:h * NCH + c + 1])

    def emit_row(s, stats_in):
        """s_i columns -> replicated row tile, via 32x32 transposes + DMA."""
        si_bf = prow.tile([128, 32], BF, tag="si_bf", name=f"si_bf_{s}")
        nc.vector.memset(si_bf[:, NCH:], 0.0)
        nc.vector.tensor_copy(si_bf[:, 0:NCH], stats_in[:, NCH:2 * NCH])
        rowt = prow.tile([32, 128], BF, tag="rowt", name=f"rowt_{s}")
        for b in range(4):
            nc.vector.transpose(rowt[0:32, b * 32:(b + 1) * 32],
                                si_bf[b * 32:(b + 1) * 32, :])
        dlin = pdram.tile([NCH, 128], BF, tag="dlin", name=f"dlin_{s}")
        nc.sync.dma_start(dlin[:, :], rowt[0:NCH, :])
        s_repl = prepl.tile([128, N], BF, tag="s_repl", name=f"s_repl_{s}")
        bcast = dlin[:, :].flatten().partition_broadcast(64)
        nc.scalar.dma_start(s_repl[0:64, :], bcast)
        nc.sync.dma_start(s_repl[64:128, :], bcast)
        return s_repl

    def emit_stats2(s, stats_in, s_repl):
        nc.vector.tensor_tensor(stats_in[:, 2 * NCH:], stats_in[:, :2 * NCH],
                                stats_in[:, :2 * NCH], ALU.mult)
        sums4 = psmall.tile([128, 4], FP, tag="sums4", name=f"sums4_{s}")
        nc.vector.tensor_reduce(
            sums4[:, :],
            stats_in[:, :].rearrange("p (g c) -> p g c", g=4),
            mybir.AxisListType.X, ALU.add)
        tot4 = psmall.tile([128, 4], FP, tag="tot4", name=f"tot4_{s}")
        nc.gpsimd.partition_all_reduce(tot4[:, :], sums4[:, :], 128,
                                       bass_isa.ReduceOp.add)
        mean4 = psmall.tile([128, 4], FP, tag="mean4", name=f"mean4_{s}")
        nc.vector.tensor_scalar(mean4[:, :], tot4[:, :], 1.0 / N, None, ALU.mult)
        m = psmall.tile([128, 1], FP, tag="m", name=f"m_{s}")
        nc.vector.tensor_tensor(m[:, :], mean4[:, 0:1], mean4[:, 1:2], ALU.add)
        msq = psmall.tile([128, 2], FP, tag="msq", name=f"msq_{s}")
        nc.vector.tensor_tensor(msq[:, :], mean4[:, 0:2], mean4[:, 0:2], ALU.mult)
        q = psmall.tile([128, 1], FP, tag="q", name=f"q_{s}")
        nc.vector.tensor_tensor(q[:, :], mean4[:, 2:3], mean4[:, 3:4], ALU.add)
        m2 = psmall.tile([128, 1], FP, tag="m2", name=f"m2_{s}")
        nc.vector.tensor_tensor(m2[:, :], msq[:, 0:1], msq[:, 1:2], ALU.add)
        v = psmall.tile([128, 1], FP, tag="v", name=f"v_{s}")
        nc.vector.tensor_tensor(v[:, :], q[:, :], m2[:, :], ALU.subtract)
        r = _emit_rsqrt(nc, pnwt, v[:, :])
        cc = psmall.tile([128, NCH], FP, tag="cc", name=f"cc_{s}")
        nc.vector.tensor_scalar(cc[:, :], stats_in[:, 0:NCH], m[:, 0:1], None,
                                ALU.subtract)
        # r-scaled build quantities: the build chain depends on r so the
        # scheduler cannot interleave builds into the Newton dep chain
        nb08 = psmall.tile([128, NCH], FP, tag="nb08", name=f"nb08_{s}")
        nc.vector.tensor_scalar(nb08[:, :], cc[:, :], -(1.0 - NEG), r[:, 0:1],
                                ALU.mult, ALU.mult)
        rcsh = psmall.tile([128, NCH], FP, tag="rcsh", name=f"rcsh_{s}")
        nc.vector.tensor_scalar(rcsh[:, :], cc[:, :], r[:, 0:1], BIAS_CONST,
                                ALU.mult, ALU.add)
        sr_repl = prepl.tile([128, N], BF, tag="sr_repl", name=f"sr_repl_{s}")
        nc.vector.tensor_scalar(sr_repl[:, :], s_repl[:, :], r[:, 0:1], None,
                                ALU.mult)
        st = dict(r=r, m=m, rcsh=rcsh, nb08=nb08, s_repl=sr_repl)
        if K_B > 0:
            r02 = psmall.tile([128, 1], FP, tag="r02", name=f"r02_{s}")
            nc.vector.tensor_scalar(r02[:, :], r[:, :], NEG, None, ALU.mult)
            bsc = psmall.tile([128, NCH], FP, tag="bsc", name=f"bsc_{s}")
            nc.scalar.activation(bsc[:, :], cc[:, :], AF.Exp,
                                 bias=BIAS_CONST, scale=r[:, 0:1])
            btsc = psmall.tile([128, NCH], FP, tag="btsc", name=f"btsc_{s}")
            nc.scalar.activation(btsc[:, :], cc[:, :], AF.Exp,
                                 bias=BIAS_CONST, scale=r02[:, 0:1])
            a_repl = prepl.tile([128, N], BF, tag="a_repl", name=f"a_repl_{s}")
            nc.scalar.activation(a_repl[:, :], sr_repl[:, :], AF.Exp,
                                 bias=0.0, scale=1.0)
            at_repl = prepl.tile([128, N], BF, tag="at_repl",
                                 name=f"at_repl_{s}")
            nc.scalar.activation(at_repl[:, :], sr_repl[:, :], AF.Exp,
                                 bias=0.0, scale=NEG)
            st.update(bsc=bsc, btsc=btsc, a_repl=a_repl, at_repl=at_repl)
        return st

    # ---- startup: sample 0 (and 1) loads, then s0 matvec/stats ----
    st_in = {0: psmall.tile([128, 4 * NCH], FP, tag="stats_in", name="si0")}
    xg00 = px.tile([128, 4 * D], FP, tag="xgrp0", name="xg_0_0")
    src00 = x_d[0].rearrange("(c p) d -> p c d", p=128)
    nc.sync.dma_start(xg00[:, 0:D].rearrange("p (c d) -> p c d", c=1),
                      src00[:, 0:1, :])
    nc.scalar.dma_start(xg00[:, D:].rearrange("p (c d) -> p c d", c=3),
                        src00[:, 1:4, :])
    del src00
    xgrp[(0, 0)] = xg00
    for g in range(1, 4):
        emit_xload_group(0, g, eng=(nc.sync if g % 2 == 1 else nc.scalar))
    for c in range(4):
        emit_matvec(0, c, 1, st_in[0])
        emit_matvec(0, c, 0, st_in[0])
    for c in range(4, NCH):
        emit_matvec(0, c, 1, st_in[0])
    s_repl0 = emit_row(0, st_in[0])
    for c in range(4, NCH):
        emit_matvec(0, c, 0, st_in[0])
    state = {0: emit_stats2(0, st_in[0], s_repl0)}
    row_tmp = {}

    for s in range(BL):
        po4 = [ppsum.tile([128, N // 2], FP, tag=f"po{k}",
                          name=f"po_{s}_{k}") for k in range(4)]
        den = psmall.tile([128, NCH], FP, tag="den", name=f"den_{s}")
        dinv = psmall.tile([128, NCH], FP, tag="dinv", name=f"dinv_{s}")
        stv = state[s]
        if s + 1 < BL:
            st_in[s + 1] = psmall.tile([128, 4 * NCH], FP, tag="stats_in",
                                       name=f"si{s + 1}")
        pool_v0a = {}
        pv_done = set()

        def emit_pool_v0a_half(cv, hh):
            pv_done.add((cv, hh))
            if hh == 0:
                pool_v0a[cv] = pv0p.tile([128, N], BF, tag="pv0a",
                                         name=f"pv0a_{s}_{cv}")
            t = pool_v0a[cv]
            hsl = slice(hh * (N // 2), (hh + 1) * (N // 2))
            nc.gpsimd.tensor_scalar(t[:, hsl], stv["s_repl"][:, hsl], NEG,
                                    stv["nb08"][:, cv:cv + 1],
                                    ALU.mult, ALU.add)

        for c in range(NCH):
            # ---- current chunk build -> et (bf16) ----
            et = pet.tile([128, N], BF, tag="et", name=f"et_{s}_{c}")
            if c < NA:
                if c in POOL_V0A and s + 1 < BL:
                    v0a = pool_v0a.pop(c)
                else:
                    v0a = pv0.tile([128, N], BF, tag="v0a",
                                   name=f"v0a_{s}_{c}")
                    nc.vector.tensor_scalar(v0a[:, :], stv["s_repl"][:, :],
                                            NEG, stv["nb08"][:, c:c + 1],
                                            ALU.mult, ALU.add)
                v0 = pv0.tile([128, N], BF, tag="v0", name=f"v0_{s}_{c}")
                nc.vector.tensor_tensor(v0[:, :], v0a[:, :],
                                        stv["s_repl"][:, :], ALU.max)
                nc.scalar.activation(
                    et[:, :], v0[:, :], AF.Exp,
                    bias=stv["rcsh"][:, c:c + 1], scale=1.0,
                    accum_out=den[:, c:c + 1])
            else:
                u = pv0.tile([128, N], BF, tag="v0a", name=f"u_{s}_{c}")
                nc.vector.tensor_scalar(u[:, :], stv["a_repl"][:, :],
                                        stv["bsc"][:, c:c + 1], None, ALU.mult)
                vv = pv0.tile([128, N], BF, tag="v0", name=f"v_{s}_{c}")
                nc.vector.tensor_scalar(vv[:, :], stv["at_repl"][:, :],
                                        stv["btsc"][:, c:c + 1], None, ALU.mult)
                nc.vector.tensor_tensor_reduce(
                    et[:, :], u[:, :], vv[:, :], 1.0, 0.0,
                    ALU.max, ALU.add, den[:, c:c + 1])
            # ---- den -> dinv -> xt (bf16 on Pool) -> matmuls ----
            nc.vector.reciprocal(dinv[:, c:c + 1], den[:, c:c + 1])
            xt = pxt.tile([128, D], BF, tag="xt", name=f"xt_{s}_{c}")
            nc.gpsimd.tensor_scalar(xt[:, :], xck(s, c), dinv[:, c:c + 1],
                                    None, ALU.mult)
            for d in range(2):
                for nn in range(4):
                    k = 2 * d + nn // 2
                    nc.tensor.matmul(
                        po4[k][:, (nn % 2) * 512:(nn % 2 + 1) * 512],
                        xt[:, d * 128:(d + 1) * 128],
                        et[:, nn * 512:(nn + 1) * 512],
                        start=(c == 0), stop=(c == NCH - 1))
            if s + 1 < BL:
                for cv in POOL_V0A:
                    if cv <= c + 5 and (cv, 0) not in pv_done:
                        emit_pool_v0a_half(cv, 0)
                    if cv <= c + 4 and (cv, 1) not in pv_done:
                        emit_pool_v0a_half(cv, 1)
            # ---- interleaved next-sample prologue (after chunk work;
            #      delayed so s0's Newton chain runs without DVE backfill) ----
            if s + 1 < BL:
                if c == 0:
                    for g in range(4):
                        tok = px.tile([128, 4 * D], FP,
                                      tag=f"xgrp{(s + 1) % 2}",
                                      name=f"tok_{s}_{g}")
                        nc.vector.tensor_copy(tok[:, 0:1], stv["m"][:, 0:1])
                elif c == 1:
                    for g in range(4):
                        emit_xload_group(s + 1, g)
                elif 3 <= c < 7:
                    for k in range(4):
                        emit_matvec(s + 1, 4 * (c - 3) + k, 1, st_in[s + 1])
                elif 7 <= c < 11:
                    for k in range(4):
                        emit_matvec(s + 1, 4 * (c - 7) + k, 0, st_in[s + 1])
                elif c == 12:
                    row_tmp[s + 1] = emit_row(s + 1, st_in[s + 1])
                elif c == 13:
                    state[s + 1] = emit_stats2(s + 1, st_in[s + 1],
                                               row_tmp[s + 1])

        # ---- drain: tanh on ACT (same table set as Exp), affine on Pool ----
        stg_t = [pstg.tile([128, N], FP, tag="stg_t", name=f"stgt_{s}_{d}")
                 for d in range(2)]
        stg = [pstg.tile([128, N], BF, tag="stg", name=f"stg_{s}_{d}")
               for d in range(2)]
        for k in range(4):
            d, h = divmod(k, 2)
            sl = slice(h * (N // 2), (h + 1) * (N // 2))
            nc.scalar.activation(stg_t[d][:, sl], po4[k][:, :], AF.Tanh,
                                 bias=0.0, scale=0.5)
            nc.vector.tensor_scalar(stg[d][:, sl], stg_t[d][:, sl], 0.5, 0.5,
                                    ALU.mult, ALU.add)
            dq = nc.sync if k % 2 == 0 else nc.scalar
            dq.dma_start(out_d[s, d * 128:(d + 1) * 128, sl], stg[d][:, sl])

    for p in reversed(ctxs):
        p.release()


_NC = {}


def _get_nc(reps=1):
    if reps not in _NC:
        nc = bacc.Bacc("TRN2", target_bir_lowering=False, debug=False,
                       enable_asserts=False, num_devices=NCORES)
        x_d = nc.dram_tensor("x", [BL, N, D], FP, kind="ExternalInput").ap()
        w_d = nc.dram_tensor("w", [128, 2 * D], FP, kind="ExternalInput").ap()
        out_d = nc.dram_tensor("out_t", [BL, D, N], BF, kind="ExternalOutput").ap()
        with tile.TileContext(nc) as tc:
            for _ in range(reps):
                _emit_body(tc, out_d, x_d, w_d)
        nc.compile()
        _NC[reps] = nc
    return _NC[reps]


def _numpy_fallback(x, weight, gamma, beta):
    out = np.empty((x.shape[0], x.shape[1], x.shape[2]), np.float32)
    d = x.shape[-1]
    for b in range(x.shape[0]):
        xb = x[b].astype(np.float64)
        s_j = xb @ weight[:d].astype(np.float64)
        s_i = xb @ weight[d:].astype(np.float64)
        att = s_i[:, None] + s_j[None, :]
        mean = att.mean()
        var = ((att - mean) ** 2).mean()
        att = (att - mean) / np.sqrt(var + EPS) * gamma + beta
        att = np.where(att >= 0, att, NEG * att)
        att = att - att.max(axis=0, keepdims=True)
        e = np.exp(att)
        att = e / e.sum(axis=0, keepdims=True)
        out[b] = 1.0 / (1.0 + np.exp(-(att @ xb)))
    return out


def run(inputs, trace=False):
    """Run the device kernel. Returns (output, exec_time_ns or None)."""
    x = np.ascontiguousarray(np.asarray(inputs["x"], dtype=np.float32))
    w = np.asarray(inputs["weight"], dtype=np.float32)
    w_repl = np.ascontiguousarray(np.broadcast_to(w, (128, 2 * D)))
    nc = _get_nc()
    in_maps = [
        {"x": np.ascontiguousarray(x[i * BL:(i + 1) * BL]), "w": w_repl}
        for i in range(NCORES)
    ]
    try:
        res = run_bass_kernel_spmd(nc, in_maps, core_ids=list(range(NCORES)),
                                   trace=trace)
    except ModuleNotFoundError:
        res = run_bass_kernel_spmd(nc, in_maps, core_ids=list(range(NCORES)),
                                   trace=False)
    parts = [np.transpose(res.results[i]["out_t"].astype(np.float32),
                          (0, 2, 1))
             for i in range(NCORES)]
    out = np.concatenate(parts, axis=0)
    return out, res.exec_time_ns


def kernel(**inputs):
    gamma = np.asarray(inputs["gamma"])
    beta = np.asarray(inputs["beta"])
    if not (np.all(gamma == 1.0) and np.all(beta == 0.0)):
        return _numpy_fallback(
            np.asarray(inputs["x"], np.float32),
            np.asarray(inputs["weight"], np.float32),
            gamma.astype(np.float32), beta.astype(np.float32))
    out, _ = run(inputs)
    return out


# revision 32
# speedup vs baseline: 1.2255x; 1.0016x over previous
"""Trainium2 Bass kernel for nn_GAT_78546361909763.

Computes, per sample b (B=16, N=2048, D=256):
    s_j = x @ w[:D];  s_i = x @ w[D:]
    att[i,j] = s_i[i] + s_j[j]
    att = LayerNorm_{(N,N)}(att) * gamma + beta    (gamma==1, beta==0 fast path)
    att = LeakyReLU_{0.2}(att)
    att = softmax(att, axis=-2)                     (normalize each column j over i)
    out = sigmoid(att @ x)

Algebraic structure exploited on-device:
  * LayerNorm stats over the (N,N) matrix decompose: mean = mean(s_i)+mean(s_j),
    var = var(s_i)+var(s_j), so stats come from the two (N,) vectors.
  * exp(leaky(z)) with z = r*(s+c) equals exp(r*(max(s, 0.2*s - 0.8*c) + c)):
    one DVE tensor_scalar + one tensor_tensor max + one ACT Exp per tile.
  * Factorized alternative (no ACT): exp(r*max(a,b)) = max(exp(ra), exp(rb)),
    so et = max(B_j*A_i, Bt_j*At_i) with A = exp(r*s_i) etc. — two DVE
    tensor_scalar multiplies + one tensor_tensor_reduce max (accumulating den).
  * The softmax denominator depends only on the contraction index j, so it
    folds into x:  out[i,d] = sum_j expT[j,i] * (x[j,d] / den[j]).
  * Per-j scale factors cancel between et and den, so each chunk may carry an
    arbitrary exponent shift: a constant -SHIFT keeps exp() inside fp8e4m3
    range, enabling fp8 DoubleRow matmuls (2 j-chunks contracted per pass).
  * fp8 stores truncate; pre-scaling by F8CORR=1+2^-5 centers the error.

Layout: att is built transposed (j on partitions, i on the free axis): the
softmax reduction is a free-axis accumulation (ACT accum_out / TTR accum) and
out_T[d,i] = sum_j xt[j,d] * expT[j,i] contracts j on partitions. The kernel
emits out_T (BL, D, N); the host transposes back.

Sharding: data-parallel over B across 8 cores (2 samples per core).
"""

import sys

sys.path.insert(0, "/opt/trn_rl_repo")

import math

import numpy as np

import concourse.bass as bass
import concourse.tile as tile
from concourse import bacc, bass_isa, mybir
from concourse.bass_utils import run_bass_kernel_spmd

B, N, D = 16, 2048, 256
NCORES = 8
BL = B // NCORES            # samples per core
NCH = N // 128              # 16 row chunks of 128
NEG = 0.2                   # leaky relu slope
EPS = 1e-14
FP = mybir.dt.float32
BF = mybir.dt.bfloat16
F8 = mybir.dt.float8e4
AF = mybir.ActivationFunctionType
ALU = mybir.AluOpType
PM = mybir.MatmulPerfMode

BIAS_CONST = 0.0            # bf16 et needs no range shift
K_B = 0                     # factorized (no-ACT) chunks per sample (at the end)
NA = NCH - K_B
POOL_V0A = frozenset((1, 3, 5, 7, 9, 11, 13, 15))   # chunks whose v0a build runs on Pool


def _emit_rsqrt(nc, pool, v_ap):
    """r = 1/sqrt(v + EPS) on DVE only: fast-inverse-sqrt seed + 3 Newton."""
    vv = pool.tile([128, 1], FP, tag="nwt_vv")
    nc.vector.tensor_scalar(vv[:, :], v_ap, float(EPS), None, ALU.add)
    yi = pool.tile([128, 1], mybir.dt.int32, tag="nwt_yi")
    nc.vector.tensor_scalar(yi[:, :], vv[:, :].bitcast(mybir.dt.int32), 1, None,
                            ALU.arith_shift_right)
    nc.vector.tensor_scalar(yi[:, :], yi[:, :], -1, 0x5F3759DF,
                            ALU.mult, ALU.add)
    y = pool.tile([128, 1], FP, tag="nwt_y")
    nc.vector.tensor_copy(y[:, :], yi[:, :].bitcast(FP))
    t = pool.tile([128, 1], FP, tag="nwt_t")
    for _ in range(2):
        nc.vector.tensor_tensor(t[:, :], y[:, :], y[:, :], ALU.mult)
        nc.vector.tensor_tensor(t[:, :], t[:, :], vv[:, :], ALU.mult)
        nc.vector.tensor_scalar(t[:, :], t[:, :], -0.5, 1.5, ALU.mult, ALU.add)
        nc.vector.tensor_tensor(y[:, :], y[:, :], t[:, :], ALU.mult)
    return y


def _emit_body(tc, out_d, x_d, w_d):
    nc = tc.nc
    ctxs = []

    def mkpool(name, bufs, **kw):
        p = tc.alloc_tile_pool(name=name, bufs=bufs, **kw)
        ctxs.append(p)
        return p

    consts = mkpool("consts", 1)
    px = mkpool("px", 4)             # x group tiles f32 [128, 4*D]
    pscr = mkpool("pscr", 8)         # matvec product scratch
    psmall = mkpool("psmall", 2)     # per-sample small tiles
    pnwt = mkpool("pnwt", 2)         # newton temps
    prepl = mkpool("prepl", 2)       # s_repl / A_repl / At_repl
    prow = mkpool("prow", 2)         # transpose staging
    pv0 = mkpool("pv0", 7)           # build tiles bf16 [128, N]
    pv0p = mkpool("pv0p", 3)         # Pool-built v0a lookahead tiles
    pet = mkpool("pet", 8)           # exp tiles bf16 [128, N]
    pxt = mkpool("pxt", 4)           # x~ tiles bf16 [128, D]
    pstg = mkpool("pstg", 2)         # output staging f32 [128, N]
    ppsum = mkpool("ppsum", 1, space="PSUM")
    pdram = mkpool("pdram", 2, space="DRAM")

    w_sb = consts.tile([128, 2 * D], FP)
    nc.sync.dma_start(w_sb[:, D:], w_d[:, D:])
    nc.scalar.dma_start(w_sb[:, 0:D], w_d[:, 0:D])

    xgrp = {}         # (s, g) -> x group tiles [128, 4, D] f32

    def emit_xload_group(s, g, eng=None):
        xg = px.tile([128, 4 * D], FP, tag=f"xgrp{s % 2}", name=f"xg_{s}_{g}")
        src = x_d[s].rearrange("(c p) d -> p c d", p=128)[:, 4 * g:4 * g + 4, :]
        (eng or nc.sync).dma_start(
            xg[:, :].rearrange("p (c d) -> p c d", c=4), src)
        xgrp[(s, g)] = xg

    def xck(s, c):
        return xgrp[(s, c // 4)][:, (c % 4) * D:(c % 4 + 1) * D]

    def emit_matvec(s, c, h, stats_in):
        scr = pscr.tile([128, D], FP, tag="scr", name=f"scr_{s}_{c}_{h}")
        nc.vector.scalar_tensor_tensor(
            scr[:, :], xck(s, c), 0.0, w_sb[:, h * D:(h + 1) * D],
            ALU.bypass, ALU.mult,
            accum_out=stats_in[:, h * NCH + c:h * NCH + c + 1])

    def emit_row(s, stats_in):
        """s_i columns -> replicated row tile, via 32x32 transposes + DMA."""
        si_bf = prow.tile([128, 32], BF, tag="si_bf", name=f"si_bf_{s}")
        nc.vector.memset(si_bf[:, NCH:], 0.0)
        nc.vector.tensor_copy(si_bf[:, 0:NCH], stats_in[:, NCH:2 * NCH])
        rowt = prow.tile([32, 128], BF, tag="rowt", name=f"rowt_{s}")
        for b in range(4):
            nc.vector.transpose(rowt[0:32, b * 32:(b + 1) * 32],
                                si_bf[b * 32:(b + 1) * 32, :])
        dlin = pdram.tile([NCH, 128], BF, tag="dlin", name=f"dlin_{s}")
        nc.sync.dma_start(dlin[:, :], rowt[0:NCH, :])
        s_repl = prepl.tile([128, N], BF, tag="s_repl", name=f"s_repl_{s}")
        bcast = dlin[:, :].flatten().partition_broadcast(64)
        nc.scalar.dma_start(s_repl[0:64, :], bcast)
        nc.sync.dma_start(s_repl[64:128, :], bcast)
        return s_repl

    def emit_stats2(s, stats_in, s_repl):
        nc.vector.tensor_tensor(stats_in[:, 2 * NCH:], stats_in[:, :2 * NCH],
                                stats_in[:, :2 * NCH], ALU.mult)
        sums4 = psmall.tile([128, 4], FP, tag="sums4", name=f"sums4_{s}")
        nc.vector.tensor_reduce(
            sums4[:, :],
            stats_in[:, :].rearrange("p (g c) -> p g c", g=4),
            mybir.AxisListType.X, ALU.add)
        tot4 = psmall.tile([128, 4], FP, tag="tot4", name=f"tot4_{s}")
        nc.gpsimd.partition_all_reduce(tot4[:, :], sums4[:, :], 128,
                                       bass_isa.ReduceOp.add)
        mean4 = psmall.tile([128, 4], FP, tag="mean4", name=f"mean4_{s}")
        nc.vector.tensor_scalar(mean4[:, :], tot4[:, :], 1.0 / N, None, ALU.mult)
        m = psmall.tile([128, 1], FP, tag="m", name=f"m_{s}")
        nc.vector.tensor_tensor(m[:, :], mean4[:, 0:1], mean4[:, 1:2], ALU.add)
        msq = psmall.tile([128, 2], FP, tag="msq", name=f"msq_{s}")
        nc.vector.tensor_tensor(msq[:, :], mean4[:, 0:2], mean4[:, 0:2], ALU.mult)
        q = psmall.tile([128, 1], FP, tag="q", name=f"q_{s}")
        nc.vector.tensor_tensor(q[:, :], mean4[:, 2:3], mean4[:, 3:4], ALU.add)
        m2 = psmall.tile([128, 1], FP, tag="m2", name=f"m2_{s}")
        nc.vector.tensor_tensor(m2[:, :], msq[:, 0:1], msq[:, 1:2], ALU.add)
        v = psmall.tile([128, 1], FP, tag="v", name=f"v_{s}")
        nc.vector.tensor_tensor(v[:, :], q[:, :], m2[:, :], ALU.subtract)
        r = _emit_rsqrt(nc, pnwt, v[:, :])
        cc = psmall.tile([128, NCH], FP, tag="cc", name=f"cc_{s}")
        nc.vector.tensor_scalar(cc[:, :], stats_in[:, 0:NCH], m[:, 0:1], None,
                                ALU.subtract)
        # r-scaled build quantities: the build chain depends on r so the
        # scheduler cannot interleave builds into the Newton dep chain
        nb08 = psmall.tile([128, NCH], FP, tag="nb08", name=f"nb08_{s}")
        nc.vector.tensor_scalar(nb08[:, :], cc[:, :], -(1.0 - NEG), r[:, 0:1],
                                ALU.mult, ALU.mult)
        rcsh = psmall.tile([128, NCH], FP, tag="rcsh", name=f"rcsh_{s}")
        nc.vector.tensor_scalar(rcsh[:, :], cc[:, :], r[:, 0:1], BIAS_CONST,
                                ALU.mult, ALU.add)
        sr_repl = prepl.tile([128, N], BF, tag="sr_repl", name=f"sr_repl_{s}")
        nc.vector.tensor_scalar(sr_repl[:, :], s_repl[:, :], r[:, 0:1], None,
                                ALU.mult)
        st = dict(r=r, m=m, rcsh=rcsh, nb08=nb08, s_repl=sr_repl)
        if K_B > 0:
            r02 = psmall.tile([128, 1], FP, tag="r02", name=f"r02_{s}")
            nc.vector.tensor_scalar(r02[:, :], r[:, :], NEG, None, ALU.mult)
            bsc = psmall.tile([128, NCH], FP, tag="bsc", name=f"bsc_{s}")
            nc.scalar.activation(bsc[:, :], cc[:, :], AF.Exp,
                                 bias=BIAS_CONST, scale=r[:, 0:1])
            btsc = psmall.tile([128, NCH], FP, tag="btsc", name=f"btsc_{s}")
            nc.scalar.activation(btsc[:, :], cc[:, :], AF.Exp,
                                 bias=BIAS_CONST, scale=r02[:, 0:1])
            a_repl = prepl.tile([128, N], BF, tag="a_repl", name=f"a_repl_{s}")
            nc.scalar.activation(a_repl[:, :], sr_repl[:, :], AF.Exp,
                                 bias=0.0, scale=1.0)
            at_repl = prepl.tile([128, N], BF, tag="at_repl",
                                 name=f"at_repl_{s}")
            nc.scalar.activation(at_repl[:, :], sr_repl[:, :], AF.Exp,
                                 bias=0.0, scale=NEG)
            st.update(bsc=bsc, btsc=btsc, a_repl=a_repl, at_repl=at_repl)
        return st

    # ---- startup: sample 0 (and 1) loads, then s0 matvec/stats ----
    st_in = {0: psmall.tile([128, 4 * NCH], FP, tag="stats_in", name="si0")}
    xg00 = px.tile([128, 4 * D], FP, tag="xgrp0", name="xg_0_0")
    src00 = x_d[0].rearrange("(c p) d -> p c d", p=128)
    nc.sync.dma_start(xg00[:, 0:D].rearrange("p (c d) -> p c d", c=1),
                      src00[:, 0:1, :])
    nc.scalar.dma_start(xg00[:, D:].rearrange("p (c d) -> p c d", c=3),
                        src00[:, 1:4, :])
    del src00
    xgrp[(0, 0)] = xg00
    for g in range(1, 4):
        emit_xload_group(0, g, eng=(nc.sync if g % 2 == 1 else nc.scalar))
    for c in range(4):
        emit_matvec(0, c, 1, st_in[0])
        emit_matvec(0, c, 0, st_in[0])
    for c in range(4, NCH):
        emit_matvec(0, c, 1, st_in[0])
    s_repl0 = emit_row(0, st_in[0])
    for c in range(4, NCH):
        emit_matvec(0, c, 0, st_in[0])
    state = {0: emit_stats2(0, st_in[0], s_repl0)}
    row_tmp = {}

    for s in range(BL):
        po4 = [ppsum.tile([128, N // 2], FP, tag=f"po{k}",
                          name=f"po_{s}_{k}") for k in range(4)]
        den = psmall.tile([128, NCH], FP, tag="den", name=f"den_{s}")
        dinv = psmall.tile([128, NCH], FP, tag="dinv", name=f"dinv_{s}")
        stv = state[s]
        if s + 1 < BL:
            st_in[s + 1] = psmall.tile([128, 4 * NCH], FP, tag="stats_in",
                                       name=f"si{s + 1}")
        pool_v0a = {}
        pv_done = set()

        def emit_pool_v0a_half(cv, hh):
            pv_done.add((cv, hh))
            if hh == 0:
                pool_v0a[cv] = pv0p.tile([128, N], BF, tag="pv0a",
                                         name=f"pv0a_{s}_{cv}")
            t = pool_v0a[cv]
            hsl = slice(hh * (N // 2), (hh + 1) * (N // 2))
            nc.gpsimd.tensor_scalar(t[:, hsl], stv["s_repl"][:, hsl], NEG,
                                    stv["nb08"][:, cv:cv + 1],
                                    ALU.mult, ALU.add)

        for c in range(NCH):
            # ---- current chunk build -> et (bf16) ----
            et = pet.tile([128, N], BF, tag="et", name=f"et_{s}_{c}")
            if c < NA:
                if c in POOL_V0A and s + 1 < BL:
                    v0a = pool_v0a.pop(c)
                else:
                    v0a = pv0.tile([128, N], BF, tag="v0a",
                                   name=f"v0a_{s}_{c}")
                    nc.vector.tensor_scalar(v0a[:, :], stv["s_repl"][:, :],
                                            NEG, stv["nb08"][:, c:c + 1],
                                            ALU.mult, ALU.add)
                v0 = pv0.tile([128, N], BF, tag="v0", name=f"v0_{s}_{c}")
                nc.vector.tensor_tensor(v0[:, :], v0a[:, :],
                                        stv["s_repl"][:, :], ALU.max)
                nc.scalar.activation(
                    et[:, :], v0[:, :], AF.Exp,
                    bias=stv["rcsh"][:, c:c + 1], scale=1.0,
                    accum_out=den[:, c:c + 1])
            else:
                u = pv0.tile([128, N], BF, tag="v0a", name=f"u_{s}_{c}")
                nc.vector.tensor_scalar(u[:, :], stv["a_repl"][:, :],
                                        stv["bsc"][:, c:c + 1], None, ALU.mult)
                vv = pv0.tile([128, N], BF, tag="v0", name=f"v_{s}_{c}")
                nc.vector.tensor_scalar(vv[:, :], stv["at_repl"][:, :],
                                        stv["btsc"][:, c:c + 1], None, ALU.mult)
                nc.vector.tensor_tensor_reduce(
                    et[:, :], u[:, :], vv[:, :], 1.0, 0.0,
                    ALU.max, ALU.add, den[:, c:c + 1])
            # ---- den -> dinv -> xt (bf16 on Pool) -> matmuls ----
            nc.vector.reciprocal(dinv[:, c:c + 1], den[:, c:c + 1])
            xt = pxt.tile([128, D], BF, tag="xt", name=f"xt_{s}_{c}")
            nc.gpsimd.tensor_scalar(xt[:, :], xck(s, c), dinv[:, c:c + 1],
                                    None, ALU.mult)
            for d in range(2):
                for nn in range(4):
                    k = 2 * d + nn // 2
                    nc.tensor.matmul(
                        po4[k][:, (nn % 2) * 512:(nn % 2 + 1) * 512],
                        xt[:, d * 128:(d + 1) * 128],
                        et[:, nn * 512:(nn + 1) * 512],
                        start=(c == 0), stop=(c == NCH - 1))
            if s + 1 < BL:
                for cv in POOL_V0A:
                    if cv <= c + 5 and (cv, 0) not in pv_done:
                        emit_pool_v0a_half(cv, 0)
                    if cv <= c + 4 and (cv, 1) not in pv_done:
                        emit_pool_v0a_half(cv, 1)
            # ---- interleaved next-sample prologue (after chunk work;
            #      delayed so s0's Newton chain runs without DVE backfill) ----
            if s + 1 < BL:
                if c == 0:
                    for g in range(4):
                        tok = px.tile([128, 4 * D], FP,
                                      tag=f"xgrp{(s + 1) % 2}",
                                      name=f"tok_{s}_{g}")
                        nc.vector.tensor_copy(tok[:, 0:1], stv["m"][:, 0:1])
                elif c == 1:
                    for g in range(4):
                        emit_xload_group(s + 1, g)
                elif 3 <= c < 7:
                    for k in range(4):
                        emit_matvec(s + 1, 4 * (c - 3) + k, 1, st_in[s + 1])
                elif 7 <= c < 11:
                    for k in range(4):
                        emit_matvec(s + 1, 4 * (c - 7) + k, 0, st_in[s + 1])
                elif c == 12:
                    row_tmp[s + 1] = emit_row(s + 1, st_in[s + 1])
                elif c == 13:
                    state[s + 1] = emit_stats2(s + 1, st_in[s + 1],
                                               row_tmp[s + 1])

        # ---- drain: tanh on ACT (same table set as Exp), affine on Pool ----
        stg_t = [pstg.tile([128, N], FP, tag="stg_t", name=f"stgt_{s}_{d}")
                 for d in range(2)]
        stg = [pstg.tile([128, N], BF, tag="stg", name=f"stg_{s}_{d}")
               for d in range(2)]
        for k in range(4):
            d, h = divmod(k, 2)
            base = h * (N // 2)
            if s == BL - 1 and k == 3:
                pieces = [(0, N // 4), (N // 4, N // 2)]
            else:
                pieces = [(0, N // 2)]
            for lo, hi in pieces:
                sl = slice(base + lo, base + hi)
                nc.scalar.activation(stg_t[d][:, sl], po4[k][:, lo:hi],
                                     AF.Tanh, bias=0.0, scale=0.5)
                nc.vector.tensor_scalar(stg[d][:, sl], stg_t[d][:, sl],
                                        0.5, 0.5, ALU.mult, ALU.add)
                dq = nc.sync if k % 2 == 0 else nc.scalar
                dq.dma_start(out_d[s, d * 128:(d + 1) * 128, sl],
                             stg[d][:, sl])

    for p in reversed(ctxs):
        p.release()


_NC = {}


def _get_nc(reps=1):
    if reps not in _NC:
        nc = bacc.Bacc("TRN2", target_bir_lowering=False, debug=False,
                       enable_asserts=False, num_devices=NCORES)
        x_d = nc.dram_tensor("x", [BL, N, D], FP, kind="ExternalInput").ap()
        w_d = nc.dram_tensor("w", [128, 2 * D], FP, kind="ExternalInput").ap()
        out_d = nc.dram_tensor("out_t", [BL, D, N], BF, kind="ExternalOutput").ap()
        with tile.TileContext(nc) as tc:
            for _ in range(reps):
                _emit_body(tc, out_d, x_d, w_d)
        nc.compile()
        _NC[reps] = nc
    return _NC[reps]


def _numpy_fallback(x, weight, gamma, beta):
    out = np.empty((x.shape[0], x.shape[1], x.shape[2]), np.float32)
    d = x.shape[-1]
    for b in range(x.shape[0]):
        xb = x[b].astype(np.float64)
        s_j = xb @ weight[:d].astype(np.float64)
        s_i = xb @ weight[d:].astype(np.float64)
        att = s_i[:, None] + s_j[None, :]
        mean = att.mean()
        var = ((att - mean) ** 2).mean()
        att = (att - mean) / np.sqrt(var + EPS) * gamma + beta
        att = np.where(att >= 0, att, NEG * att)
        att = att - att.max(axis=0, keepdims=True)
        e = np.exp(att)
        att = e / e.sum(axis=0, keepdims=True)
        out[b] = 1.0 / (1.0 + np.exp(-(att @ xb)))
    return out


def run(inputs, trace=False):
    """Run the device kernel. Returns (output, exec_time_ns or None)."""
    x = np.ascontiguousarray(np.asarray(inputs["x"], dtype=np.float32))
    w = np.asarray(inputs["weight"], dtype=np.float32)
    w_repl = np.ascontiguousarray(np.broadcast_to(w, (128, 2 * D)))
    nc = _get_nc()
    in_maps = [
        {"x": np.ascontiguousarray(x[i * BL:(i + 1) * BL]), "w": w_repl}
        for i in range(NCORES)
    ]
    try:
        res = run_bass_kernel_spmd(nc, in_maps, core_ids=list(range(NCORES)),
                                   trace=trace)
    except ModuleNotFoundError:
        res = run_bass_kernel_spmd(nc, in_maps, core_ids=list(range(NCORES)),
                                   trace=False)
    parts = [np.transpose(res.results[i]["out_t"].astype(np.float32),
                          (0, 2, 1))
             for i in range(NCORES)]
    out = np.concatenate(parts, axis=0)
    return out, res.exec_time_ns


def kernel(**inputs):
    gamma = np.asarray(inputs["gamma"])
    beta = np.asarray(inputs["beta"])
    if not (np.all(gamma == 1.0) and np.all(beta == 0.0)):
        return _numpy_fallback(
            np.asarray(inputs["x"], np.float32),
            np.asarray(inputs["weight"], np.float32),
            gamma.astype(np.float32), beta.astype(np.float32))
    out, _ = run(inputs)
    return out


# revision 35
# speedup vs baseline: 1.2325x; 1.0057x over previous
"""Trainium2 Bass kernel for nn_GAT_78546361909763.

Computes, per sample b (B=16, N=2048, D=256):
    s_j = x @ w[:D];  s_i = x @ w[D:]
    att[i,j] = s_i[i] + s_j[j]
    att = LayerNorm_{(N,N)}(att) * gamma + beta    (gamma==1, beta==0 fast path)
    att = LeakyReLU_{0.2}(att)
    att = softmax(att, axis=-2)                     (normalize each column j over i)
    out = sigmoid(att @ x)

Algebraic structure exploited on-device:
  * LayerNorm stats over the (N,N) matrix decompose: mean = mean(s_i)+mean(s_j),
    var = var(s_i)+var(s_j), so stats come from the two (N,) vectors.
  * exp(leaky(z)) with z = r*(s+c) equals exp(r*(max(s, 0.2*s - 0.8*c) + c)):
    one DVE tensor_scalar + one tensor_tensor max + one ACT Exp per tile.
  * Factorized alternative (no ACT): exp(r*max(a,b)) = max(exp(ra), exp(rb)),
    so et = max(B_j*A_i, Bt_j*At_i) with A = exp(r*s_i) etc. — two DVE
    tensor_scalar multiplies + one tensor_tensor_reduce max (accumulating den).
  * The softmax denominator depends only on the contraction index j, so it
    folds into x:  out[i,d] = sum_j expT[j,i] * (x[j,d] / den[j]).
  * Per-j scale factors cancel between et and den, so each chunk may carry an
    arbitrary exponent shift: a constant -SHIFT keeps exp() inside fp8e4m3
    range, enabling fp8 DoubleRow matmuls (2 j-chunks contracted per pass).
  * fp8 stores truncate; pre-scaling by F8CORR=1+2^-5 centers the error.

Layout: att is built transposed (j on partitions, i on the free axis): the
softmax reduction is a free-axis accumulation (ACT accum_out / TTR accum) and
out_T[d,i] = sum_j xt[j,d] * expT[j,i] contracts j on partitions. The kernel
emits out_T (BL, D, N); the host transposes back.

Sharding: data-parallel over B across 8 cores (2 samples per core).
"""

import sys

sys.path.insert(0, "/opt/trn_rl_repo")

import math

import numpy as np

import concourse.bass as bass
import concourse.tile as tile
from concourse import bacc, bass_isa, mybir
from concourse.bass_utils import run_bass_kernel_spmd

B, N, D = 16, 2048, 256
NCORES = 8
BL = B // NCORES            # samples per core
NCH = N // 128              # 16 row chunks of 128
NEG = 0.2                   # leaky relu slope
EPS = 1e-14
FP = mybir.dt.float32
BF = mybir.dt.bfloat16
F8 = mybir.dt.float8e4
AF = mybir.ActivationFunctionType
ALU = mybir.AluOpType
PM = mybir.MatmulPerfMode

BIAS_CONST = 0.0            # bf16 et needs no range shift
K_B = 0                     # factorized (no-ACT) chunks per sample (at the end)
NA = NCH - K_B
POOL_V0A = frozenset((1, 3, 5, 7, 9, 11, 13, 15))   # chunks whose v0a build runs on Pool


def _emit_rsqrt(nc, pool, v_ap):
    """r = 1/sqrt(v + EPS) on DVE only: fast-inverse-sqrt seed + 3 Newton."""
    vv = pool.tile([128, 1], FP, tag="nwt_vv")
    nc.vector.tensor_scalar(vv[:, :], v_ap, float(EPS), None, ALU.add)
    yi = pool.tile([128, 1], mybir.dt.int32, tag="nwt_yi")
    nc.vector.tensor_scalar(yi[:, :], vv[:, :].bitcast(mybir.dt.int32), 1, None,
                            ALU.arith_shift_right)
    nc.vector.tensor_scalar(yi[:, :], yi[:, :], -1, 0x5F3759DF,
                            ALU.mult, ALU.add)
    y = pool.tile([128, 1], FP, tag="nwt_y")
    nc.vector.tensor_copy(y[:, :], yi[:, :].bitcast(FP))
    t = pool.tile([128, 1], FP, tag="nwt_t")
    for _ in range(2):
        nc.vector.tensor_tensor(t[:, :], y[:, :], y[:, :], ALU.mult)
        nc.vector.tensor_tensor(t[:, :], t[:, :], vv[:, :], ALU.mult)
        nc.vector.tensor_scalar(t[:, :], t[:, :], -0.5, 1.5, ALU.mult, ALU.add)
        nc.vector.tensor_tensor(y[:, :], y[:, :], t[:, :], ALU.mult)
    return y


def _emit_body(tc, out_d, x_d, w_d):
    nc = tc.nc
    ctxs = []

    def mkpool(name, bufs, **kw):
        p = tc.alloc_tile_pool(name=name, bufs=bufs, **kw)
        ctxs.append(p)
        return p

    consts = mkpool("consts", 1)
    px = mkpool("px", 4)             # x group tiles f32 [128, 4*D]
    pscr = mkpool("pscr", 8)         # matvec product scratch
    psmall = mkpool("psmall", 2)     # per-sample small tiles
    pnwt = mkpool("pnwt", 2)         # newton temps
    prepl = mkpool("prepl", 2)       # s_repl / A_repl / At_repl
    prow = mkpool("prow", 2)         # transpose staging
    pv0 = mkpool("pv0", 7)           # build tiles bf16 [128, N]
    pv0p = mkpool("pv0p", 3)         # Pool-built v0a lookahead tiles
    pet = mkpool("pet", 8)           # exp tiles bf16 [128, N]
    pxt = mkpool("pxt", 4)           # x~ tiles bf16 [128, D]
    pstg = mkpool("pstg", 2)         # output staging f32 [128, N]
    ppsum = mkpool("ppsum", 1, space="PSUM")
    pdram = mkpool("pdram", 2, space="DRAM")

    w_sb = consts.tile([128, 2 * D], FP)
    nc.sync.dma_start(w_sb[:, D:], w_d[:, D:])
    nc.scalar.dma_start(w_sb[:, 0:D], w_d[:, 0:D])

    xgrp = {}         # (s, g) -> x group tiles [128, 4, D] f32

    def emit_xload_group(s, g, eng=None):
        xg = px.tile([128, 4 * D], FP, tag=f"xgrp{s % 2}", name=f"xg_{s}_{g}")
        src = x_d[s].rearrange("(c p) d -> p c d", p=128)[:, 4 * g:4 * g + 4, :]
        (eng or nc.sync).dma_start(
            xg[:, :].rearrange("p (c d) -> p c d", c=4), src)
        xgrp[(s, g)] = xg

    def xck(s, c):
        return xgrp[(s, c // 4)][:, (c % 4) * D:(c % 4 + 1) * D]

    def emit_matvec(s, c, h, stats_in):
        scr = pscr.tile([128, D], FP, tag="scr", name=f"scr_{s}_{c}_{h}")
        nc.vector.scalar_tensor_tensor(
            scr[:, :], xck(s, c), 0.0, w_sb[:, h * D:(h + 1) * D],
            ALU.bypass, ALU.mult,
            accum_out=stats_in[:, h * NCH + c:h * NCH + c + 1])

    def emit_row(s, stats_in):
        """s_i columns -> replicated row tile, via 32x32 transposes + DMA."""
        si_bf = prow.tile([128, 32], BF, tag="si_bf", name=f"si_bf_{s}")
        nc.vector.memset(si_bf[:, NCH:], 0.0)
        nc.vector.tensor_copy(si_bf[:, 0:NCH], stats_in[:, NCH:2 * NCH])
        rowt = prow.tile([32, 128], BF, tag="rowt", name=f"rowt_{s}")
        for b in range(4):
            nc.vector.transpose(rowt[0:32, b * 32:(b + 1) * 32],
                                si_bf[b * 32:(b + 1) * 32, :])
        dlin = pdram.tile([NCH, 128], BF, tag="dlin", name=f"dlin_{s}")
        nc.sync.dma_start(dlin[:, :], rowt[0:NCH, :])
        s_repl = prepl.tile([128, N], BF, tag="s_repl", name=f"s_repl_{s}")
        bcast = dlin[:, :].flatten().partition_broadcast(64)
        nc.scalar.dma_start(s_repl[0:64, :], bcast)
        nc.sync.dma_start(s_repl[64:128, :], bcast)
        return s_repl

    def emit_stats2(s, stats_in, s_repl):
        nc.vector.tensor_tensor(stats_in[:, 2 * NCH:], stats_in[:, :2 * NCH],
                                stats_in[:, :2 * NCH], ALU.mult)
        sums4 = psmall.tile([128, 4], FP, tag="sums4", name=f"sums4_{s}")
        nc.vector.tensor_reduce(
            sums4[:, :],
            stats_in[:, :].rearrange("p (g c) -> p g c", g=4),
            mybir.AxisListType.X, ALU.add)
        tot4 = psmall.tile([128, 4], FP, tag="tot4", name=f"tot4_{s}")
        nc.gpsimd.partition_all_reduce(tot4[:, :], sums4[:, :], 128,
                                       bass_isa.ReduceOp.add)
        mean4 = psmall.tile([128, 4], FP, tag="mean4", name=f"mean4_{s}")
        nc.vector.tensor_scalar(mean4[:, :], tot4[:, :], 1.0 / N, None, ALU.mult)
        m = psmall.tile([128, 1], FP, tag="m", name=f"m_{s}")
        nc.vector.tensor_tensor(m[:, :], mean4[:, 0:1], mean4[:, 1:2], ALU.add)
        msq = psmall.tile([128, 2], FP, tag="msq", name=f"msq_{s}")
        nc.vector.tensor_tensor(msq[:, :], mean4[:, 0:2], mean4[:, 0:2], ALU.mult)
        q = psmall.tile([128, 1], FP, tag="q", name=f"q_{s}")
        nc.vector.tensor_tensor(q[:, :], mean4[:, 2:3], mean4[:, 3:4], ALU.add)
        m2 = psmall.tile([128, 1], FP, tag="m2", name=f"m2_{s}")
        nc.vector.tensor_tensor(m2[:, :], msq[:, 0:1], msq[:, 1:2], ALU.add)
        v = psmall.tile([128, 1], FP, tag="v", name=f"v_{s}")
        nc.vector.tensor_tensor(v[:, :], q[:, :], m2[:, :], ALU.subtract)
        r = _emit_rsqrt(nc, pnwt, v[:, :])
        cc = psmall.tile([128, NCH], FP, tag="cc", name=f"cc_{s}")
        nc.vector.tensor_scalar(cc[:, :], stats_in[:, 0:NCH], m[:, 0:1], None,
                                ALU.subtract)
        # r-scaled build quantities: the build chain depends on r so the
        # scheduler cannot interleave builds into the Newton dep chain
        nb08 = psmall.tile([128, NCH], FP, tag="nb08", name=f"nb08_{s}")
        nc.vector.tensor_scalar(nb08[:, :], cc[:, :], -(1.0 - NEG), r[:, 0:1],
                                ALU.mult, ALU.mult)
        rcsh = psmall.tile([128, NCH], FP, tag="rcsh", name=f"rcsh_{s}")
        nc.vector.tensor_scalar(rcsh[:, :], cc[:, :], r[:, 0:1], BIAS_CONST,
                                ALU.mult, ALU.add)
        sr_repl = prepl.tile([128, N], BF, tag="sr_repl", name=f"sr_repl_{s}")
        nc.vector.tensor_scalar(sr_repl[:, :], s_repl[:, :], r[:, 0:1], None,
                                ALU.mult)
        st = dict(r=r, m=m, rcsh=rcsh, nb08=nb08, s_repl=sr_repl)
        if K_B > 0:
            r02 = psmall.tile([128, 1], FP, tag="r02", name=f"r02_{s}")
            nc.vector.tensor_scalar(r02[:, :], r[:, :], NEG, None, ALU.mult)
            bsc = psmall.tile([128, NCH], FP, tag="bsc", name=f"bsc_{s}")
            nc.scalar.activation(bsc[:, :], cc[:, :], AF.Exp,
                                 bias=BIAS_CONST, scale=r[:, 0:1])
            btsc = psmall.tile([128, NCH], FP, tag="btsc", name=f"btsc_{s}")
            nc.scalar.activation(btsc[:, :], cc[:, :], AF.Exp,
                                 bias=BIAS_CONST, scale=r02[:, 0:1])
            a_repl = prepl.tile([128, N], BF, tag="a_repl", name=f"a_repl_{s}")
            nc.scalar.activation(a_repl[:, :], sr_repl[:, :], AF.Exp,
                                 bias=0.0, scale=1.0)
            at_repl = prepl.tile([128, N], BF, tag="at_repl",
                                 name=f"at_repl_{s}")
            nc.scalar.activation(at_repl[:, :], sr_repl[:, :], AF.Exp,
                                 bias=0.0, scale=NEG)
            st.update(bsc=bsc, btsc=btsc, a_repl=a_repl, at_repl=at_repl)
        return st

    # ---- startup: sample 0 (and 1) loads, then s0 matvec/stats ----
    st_in = {0: psmall.tile([128, 4 * NCH], FP, tag="stats_in", name="si0")}
    xg00 = px.tile([128, 4 * D], FP, tag="xgrp0", name="xg_0_0")
    src00 = x_d[0].rearrange("(c p) d -> p c d", p=128)
    nc.sync.dma_start(xg00[:, 0:D].rearrange("p (c d) -> p c d", c=1),
                      src00[:, 0:1, :])
    nc.scalar.dma_start(xg00[:, D:].rearrange("p (c d) -> p c d", c=3),
                        src00[:, 1:4, :])
    del src00
    xgrp[(0, 0)] = xg00
    for g in range(1, 4):
        emit_xload_group(0, g, eng=(nc.sync if g % 2 == 1 else nc.scalar))
    for c in range(4):
        emit_matvec(0, c, 1, st_in[0])
        emit_matvec(0, c, 0, st_in[0])
    for c in range(4, NCH):
        emit_matvec(0, c, 1, st_in[0])
    s_repl0 = emit_row(0, st_in[0])
    for c in range(4, NCH):
        emit_matvec(0, c, 0, st_in[0])
    state = {0: emit_stats2(0, st_in[0], s_repl0)}
    row_tmp = {}

    for s in range(BL):
        po4 = [ppsum.tile([128, N // 2], FP, tag=f"po{k}",
                          name=f"po_{s}_{k}") for k in range(4)]
        den = psmall.tile([128, NCH], FP, tag="den", name=f"den_{s}")
        dinv = psmall.tile([128, NCH], FP, tag="dinv", name=f"dinv_{s}")
        stv = state[s]
        if s + 1 < BL:
            st_in[s + 1] = psmall.tile([128, 4 * NCH], FP, tag="stats_in",
                                       name=f"si{s + 1}")
        pool_v0a = {}
        pv_done = set()

        def emit_pool_v0a_half(cv, hh):
            pv_done.add((cv, hh))
            if hh == 0:
                pool_v0a[cv] = pv0p.tile([128, N], BF, tag="pv0a",
                                         name=f"pv0a_{s}_{cv}")
            t = pool_v0a[cv]
            hsl = slice(hh * (N // 2), (hh + 1) * (N // 2))
            nc.gpsimd.tensor_scalar(t[:, hsl], stv["s_repl"][:, hsl], NEG,
                                    stv["nb08"][:, cv:cv + 1],
                                    ALU.mult, ALU.add)

        for c in range(NCH):
            # ---- current chunk build -> et (bf16) ----
            et = pet.tile([128, N], BF, tag="et", name=f"et_{s}_{c}")
            if c < NA:
                if c in POOL_V0A and s + 1 < BL:
                    v0a = pool_v0a.pop(c)
                else:
                    v0a = pv0.tile([128, N], BF, tag="v0a",
                                   name=f"v0a_{s}_{c}")
                    nc.vector.tensor_scalar(v0a[:, :], stv["s_repl"][:, :],
                                            NEG, stv["nb08"][:, c:c + 1],
                                            ALU.mult, ALU.add)
                v0 = pv0.tile([128, N], BF, tag="v0", name=f"v0_{s}_{c}")
                nc.vector.tensor_tensor(v0[:, :], v0a[:, :],
                                        stv["s_repl"][:, :], ALU.max)
                nc.scalar.activation(
                    et[:, :], v0[:, :], AF.Exp,
                    bias=stv["rcsh"][:, c:c + 1], scale=1.0,
                    accum_out=den[:, c:c + 1])
            else:
                u = pv0.tile([128, N], BF, tag="v0a", name=f"u_{s}_{c}")
                nc.vector.tensor_scalar(u[:, :], stv["a_repl"][:, :],
                                        stv["bsc"][:, c:c + 1], None, ALU.mult)
                vv = pv0.tile([128, N], BF, tag="v0", name=f"v_{s}_{c}")
                nc.vector.tensor_scalar(vv[:, :], stv["at_repl"][:, :],
                                        stv["btsc"][:, c:c + 1], None, ALU.mult)
                nc.vector.tensor_tensor_reduce(
                    et[:, :], u[:, :], vv[:, :], 1.0, 0.0,
                    ALU.max, ALU.add, den[:, c:c + 1])
            # ---- den -> dinv -> xt (bf16 on Pool) -> matmuls ----
            nc.vector.reciprocal(dinv[:, c:c + 1], den[:, c:c + 1])
            xt = pxt.tile([128, D], BF, tag="xt", name=f"xt_{s}_{c}")
            nc.gpsimd.tensor_scalar(xt[:, :], xck(s, c), dinv[:, c:c + 1],
                                    None, ALU.mult)
            for d in range(2):
                for nn in range(4):
                    k = 2 * d + nn // 2
                    nc.tensor.matmul(
                        po4[k][:, (nn % 2) * 512:(nn % 2 + 1) * 512],
                        xt[:, d * 128:(d + 1) * 128],
                        et[:, nn * 512:(nn + 1) * 512],
                        start=(c == 0), stop=(c == NCH - 1))
            if s + 1 < BL:
                for cv in POOL_V0A:
                    if cv <= c + 5 and (cv, 0) not in pv_done:
                        emit_pool_v0a_half(cv, 0)
                    if cv <= c + 4 and (cv, 1) not in pv_done:
                        emit_pool_v0a_half(cv, 1)
            # ---- interleaved next-sample prologue (after chunk work;
            #      delayed so s0's Newton chain runs without DVE backfill) ----
            if s + 1 < BL:
                if c == 0:
                    for g in range(4):
                        tok = px.tile([128, 4 * D], FP,
                                      tag=f"xgrp{(s + 1) % 2}",
                                      name=f"tok_{s}_{g}")
                        nc.vector.tensor_copy(tok[:, 0:1], stv["m"][:, 0:1])
                elif c == 1:
                    for g in range(4):
                        emit_xload_group(s + 1, g)
                elif 3 <= c < 7:
                    for k in range(4):
                        emit_matvec(s + 1, 4 * (c - 3) + k, 1, st_in[s + 1])
                elif 7 <= c < 11:
                    for k in range(4):
                        emit_matvec(s + 1, 4 * (c - 7) + k, 0, st_in[s + 1])
                elif c == 12:
                    row_tmp[s + 1] = emit_row(s + 1, st_in[s + 1])
                elif c == 13:
                    state[s + 1] = emit_stats2(s + 1, st_in[s + 1],
                                               row_tmp[s + 1])

        # ---- drain: tanh on ACT (same table set as Exp), affine on Pool ----
        stg_t = [pstg.tile([128, N], FP, tag="stg_t", name=f"stgt_{s}_{d}")
                 for d in range(2)]
        stg = [pstg.tile([128, N], BF, tag="stg", name=f"stg_{s}_{d}")
               for d in range(2)]
        for k in range(4):
            d, h = divmod(k, 2)
            base = h * (N // 2)
            if s == BL - 1 and k == 3:
                pieces = [(0, N // 4), (N // 4, N // 2)]
            else:
                pieces = [(0, N // 2)]
            aff = nc.gpsimd if s + 1 < BL else nc.vector
            for lo, hi in pieces:
                sl = slice(base + lo, base + hi)
                nc.scalar.activation(stg_t[d][:, sl], po4[k][:, lo:hi],
                                     AF.Tanh, bias=0.0, scale=0.5)
                aff.tensor_scalar(stg[d][:, sl], stg_t[d][:, sl],
                                  0.5, 0.5, ALU.mult, ALU.add)
                dq = nc.sync if k % 2 == 0 else nc.scalar
                dq.dma_start(out_d[s, d * 128:(d + 1) * 128, sl],
                             stg[d][:, sl])

    for p in reversed(ctxs):
        p.release()


_NC = {}


def _get_nc(reps=1):
    if reps not in _NC:
        nc = bacc.Bacc("TRN2", target_bir_lowering=False, debug=False,
                       enable_asserts=False, num_devices=NCORES)
        x_d = nc.dram_tensor("x", [BL, N, D], FP, kind="ExternalInput").ap()
        w_d = nc.dram_tensor("w", [128, 2 * D], FP, kind="ExternalInput").ap()
        out_d = nc.dram_tensor("out_t", [BL, D, N], BF, kind="ExternalOutput").ap()
        with tile.TileContext(nc) as tc:
            for _ in range(reps):
                _emit_body(tc, out_d, x_d, w_d)
        nc.compile()
        _NC[reps] = nc
    return _NC[reps]


def _numpy_fallback(x, weight, gamma, beta):
    out = np.empty((x.shape[0], x.shape[1], x.shape[2]), np.float32)
    d = x.shape[-1]
    for b in range(x.shape[0]):
        xb = x[b].astype(np.float64)
        s_j = xb @ weight[:d].astype(np.float64)
        s_i = xb @ weight[d:].astype(np.float64)
        att = s_i[:, None] + s_j[None, :]
        mean = att.mean()
        var = ((att - mean) ** 2).mean()
        att = (att - mean) / np.sqrt(var + EPS) * gamma + beta
        att = np.where(att >= 0, att, NEG * att)
        att = att - att.max(axis=0, keepdims=True)
        e = np.exp(att)
        att = e / e.sum(axis=0, keepdims=True)
        out[b] = 1.0 / (1.0 + np.exp(-(att @ xb)))
    return out


def run(inputs, trace=False):
    """Run the device kernel. Returns (output, exec_time_ns or None)."""
    x = np.ascontiguousarray(np.asarray(inputs["x"], dtype=np.float32))
    w = np.asarray(inputs["weight"], dtype=np.float32)
    w_repl = np.ascontiguousarray(np.broadcast_to(w, (128, 2 * D)))
    nc = _get_nc()
    in_maps = [
        {"x": np.ascontiguousarray(x[i * BL:(i + 1) * BL]), "w": w_repl}
        for i in range(NCORES)
    ]
    try:
        res = run_bass_kernel_spmd(nc, in_maps, core_ids=list(range(NCORES)),
                                   trace=trace)
    except ModuleNotFoundError:
        res = run_bass_kernel_spmd(nc, in_maps, core_ids=list(range(NCORES)),
                                   trace=False)
    parts = [np.transpose(res.results[i]["out_t"].astype(np.float32),
                          (0, 2, 1))
             for i in range(NCORES)]
    out = np.concatenate(parts, axis=0)
    return out, res.exec_time_ns


def kernel(**inputs):
    gamma = np.asarray(inputs["gamma"])
    beta = np.asarray(inputs["beta"])
    if not (np.all(gamma == 1.0) and np.all(beta == 0.0)):
        return _numpy_fallback(
            np.asarray(inputs["x"], np.float32),
            np.asarray(inputs["weight"], np.float32),
            gamma.astype(np.float32), beta.astype(np.float32))
    out, _ = run(inputs)
    return out
